# revision 1
# baseline (speedup 1.0000x reference)
"""BiLSTM-CRF loss kernel for Trainium2, 8-core data parallel.

Per-core (batch shard of 32, both LSTM directions as independent chains):
  P0: dma_gather embeddings (bf16, transposed layout: E on partitions)
  P1: input projections x @ W_ih.T + b -> zin (bf16, DRAM scratch)
  P2: 128 LSTM steps; fwd and bwd emitted per step as separate instruction
      chains so the engines pipeline across directions; h transposed per step
      via PE into hT buffers (feature-major) feeding the next step's matmul
      lhsT and the emission matmuls
  P3: emission matmuls + gold-path dot (tensor_tensor_reduce) + exp(em)
  P4: CRF forward pass in scaled linear space with an absorbing 77th tag for
      variable lengths; final log + reductions -> per-core partial sums
Host combines the 8 partial sums into the scalar loss.
"""

import numpy as np
import ml_dtypes

import concourse.bass as bass
import concourse.mybir as mybir
from concourse.tile import TileContext
from concourse import library_config
from concourse.vector_clock import ScopedClock

N_CORES = 8
B, S, E, HD, T, V = 256, 128, 512, 256, 76, 30000
BC = B // N_CORES          # 32 batch per core
G4 = 4 * HD                # 1024 gates
TA = T + 1                 # 77 tags with absorber
NTOK = S * BC              # 4096 tokens per direction per core

dt = mybir.dt
F32, BF16, I16 = dt.float32, dt.bfloat16, dt.int16
AF = mybir.ActivationFunctionType
ALU = mybir.AluOpType

# ---------------------------------------------------------------- tile patch
# This walrus build rejects >1 sem wait on CTRL-class (Drain/NoOp)
# instructions; split the Tile tail-drain waits across preceding NOPs.
_MAX_WAITS = 1


_WAIT_LIMITS = {}


def _split_excess_waits(nc):
    """Non-DMA instructions accept only one sem wait on this walrus build;
    move excess waits onto NOPs spliced in front (same engine, same order)."""
    for f in nc.m.functions:
        stack = list(f.blocks)
        while stack:
            bb = stack.pop()
            for sub in getattr(bb, "blocks", []) or []:
                stack.append(sub)
            insts = getattr(bb, "instructions", None)
            if not insts:
                continue
            newlist = []
            changed = False
            for inst in insts:
                si = inst.sync_info
                lim = _WAIT_LIMITS.get(type(inst).__name__, 1)
                if si is not None and si.on_wait and len(si.on_wait) > lim:
                    waits = list(si.on_wait)
                    si.on_wait = waits[-lim:]
                    for w in waits[:-lim]:
                        nop = mybir.InstNoOp(
                            name=f"I-wsplit{nc.next_id()}", ins=[], outs=[],
                            engine=inst.engine,
                            sync_info=mybir.SyncInfo(on_wait=[w], on_update=[]),
                        )
                        newlist.append(nop)
                    changed = True
                newlist.append(inst)
            if changed:
                insts[:] = newlist


def _patched_drain_and_barrier(self, tick_clock, wait_clock):
    nc = self.nc
    _split_excess_waits(nc)
    nops = [nc.sync.nop(nofuse=True, hint=f"waitsplit{i}") for i in range(16)]
    drain_inst = nc.sync.drain()
    wait_clock.add_sem_waits(
        drain_inst.ins, ScopedClock({None: tick_clock.global_clock})
    )
    si = drain_inst.ins.sync_info
    if si is not None and si.on_wait and len(si.on_wait) > _MAX_WAITS:
        waits = list(si.on_wait)
        chunks = [waits[i:i + _MAX_WAITS] for i in range(0, len(waits), _MAX_WAITS)]
        si.on_wait = chunks[-1]
        assert len(chunks) - 1 <= len(nops), "too many wait chunks"
        for i, ch in enumerate(chunks[:-1]):
            ni = nops[i].ins
            if ni.sync_info is None:
                ni.sync_info = mybir.SyncInfo(on_wait=ch, on_update=[])
            else:
                ni.sync_info.on_wait = list(ni.sync_info.on_wait) + ch
    nc.all_engine_barrier()
    assert self.sems is not None
    popped = nc._tile_sem_poison_stack.pop()
    assert popped is self._sem_poison
    allsems = list(self.sems.allocated().values())
    for i in range(0, len(allsems), 8):
        nc.clear_and_free_semaphores(allsems[i:i + 8])
    nc.all_engine_barrier()


def apply_tile_patch():
    TileContext._drain_and_barrier = _patched_drain_and_barrier


# ---------------------------------------------------------------- builder
def build_nc():
    apply_tile_patch()
    nc = bass.Bass("TRN2", target_bir_lowering=False, debug=False,
                   num_devices=N_CORES)

    xt_d = nc.dram_tensor("xt", [2, 128, 4, NTOK], BF16, kind="ExternalInput")
    wih = nc.dram_tensor("wih", [2, 128, 4, G4], BF16, kind="ExternalInput")
    whh = nc.dram_tensor("whh", [2, 128, 2, G4], BF16, kind="ExternalInput")
    wout = nc.dram_tensor("wout", [128, 4, T], BF16, kind="ExternalInput")
    # per-dir combined bias b_ih+b_hh (gate-reordered), replicated over 128 rows
    biasr = nc.dram_tensor("biasr", [2, 128, G4], BF16, kind="ExternalInput")
    h0t = nc.dram_tensor("h0t", [128, 2, 2 * BC], BF16, kind="ExternalInput")
    c0 = nc.dram_tensor("c0", [2 * BC, HD], F32, kind="ExternalInput")
    ident = nc.dram_tensor("ident", [128, 96], BF16, kind="ExternalInput")
    # tables: [trans(0:76) | start(76) | end(77) | bout(78) | negkappa(79)]
    tables = nc.dram_tensor("tables", [T, 80], F32, kind="ExternalInput")
    gcnt = nc.dram_tensor("gcnt", [T, 79], F32, kind="ExternalInput")
    ohm = nc.dram_tensor("ohm", [T, NTOK], BF16, kind="ExternalInput")
    vmask = nc.dram_tensor("vmask", [T, NTOK], BF16, kind="ExternalInput")
    padrow = nc.dram_tensor("padrow", [1, NTOK], F32, kind="ExternalInput")
    absrow = nc.dram_tensor("absrow", [1, 80], F32, kind="ExternalInput")
    out_d = nc.dram_tensor("out", [1, 2], F32, kind="ExternalOutput")
    zin_d = nc.dram_tensor("zin_scratch", [2, S // 4, 128, G4], BF16,
                           kind="Internal")

    with TileContext(nc) as tc:
        with (
            tc.tile_pool(name="const", bufs=1) as cpool,
            tc.tile_pool(name="hbuf", bufs=1) as hpool,
            tc.tile_pool(name="work", bufs=3) as wpool,
            tc.tile_pool(name="state", bufs=3) as spool,
            tc.tile_pool(name="mmps", bufs=2, space="PSUM") as mmps,
            tc.tile_pool(name="zups", bufs=1, space="PSUM") as zups,
            tc.tile_pool(name="smps", bufs=2, space="PSUM") as smps,
        ):
            # ---- constants / small inputs into SBUF
            wih_sb = cpool.tile([128, 2, 4, G4], BF16)
            nc.sync.dma_start(wih_sb[:], wih.ap().rearrange("d p c g -> p d c g"))
            whh_sb = cpool.tile([128, 2, 2, G4], BF16)
            nc.sync.dma_start(whh_sb[:], whh.ap().rearrange("d p c g -> p d c g"))
            wout_sb = cpool.tile([128, 4, T], BF16)
            nc.sync.dma_start(wout_sb[:], wout[:])
            bias_sb = cpool.tile([128, 2, G4], BF16)
            nc.sync.dma_start(bias_sb[:], biasr.ap().rearrange("d p g -> p d g"))
            h0t_sb = cpool.tile([128, 2, 2 * BC], BF16)
            nc.sync.dma_start(h0t_sb[:], h0t[:])
            ident_sb = cpool.tile([128, 96], BF16)
            nc.sync.dma_start(ident_sb[:], ident[:])
            tab_sb = cpool.tile([T, 80], F32)
            nc.sync.dma_start(tab_sb[:], tables[:])
            gcnt_sb = cpool.tile([T, 79], F32)
            nc.sync.dma_start(gcnt_sb[:], gcnt[:])
            # persistent big buffers
            hts = {0: hpool.tile([128, 2, NTOK], BF16, tag="hft", name="hft"),
                   1: hpool.tile([128, 2, NTOK], BF16, tag="hbt", name="hbt")}
            em_sb = hpool.tile([TA, NTOK], F32, tag="em")

            # ---- P0 + P1 in a released pool
            with tc.tile_pool(name="xg", bufs=1) as xpool:
                xg = {0: xpool.tile([128, 4, NTOK], BF16, tag="xg0", name="xg0"),
                      1: xpool.tile([128, 4, NTOK], BF16, tag="xg1", name="xg1")}
                for d in range(2):
                    nc.sync.dma_start(xg[d][:], xt_d.ap()[d])

                # token block of 128 = 4 steps; PSUM [128, 512] x2 slices
                for d in range(2):
                    for tb in range(NTOK // 128):     # 32 blocks
                        stg = wpool.tile([128, G4], BF16, tag="zstage")
                        for sl in range(2):
                            ps = mmps.tile([128, 512], F32, tag="mm")
                            for c in range(4):
                                nc.tensor.matmul(
                                    ps[:],
                                    xg[d][:, c, tb * 128:(tb + 1) * 128],
                                    wih_sb[:, d, c, sl * 512:(sl + 1) * 512],
                                    start=(c == 0), stop=(c == 3),
                                )
                            nc.vector.tensor_add(
                                stg[:, sl * 512:(sl + 1) * 512], ps[:],
                                bias_sb[:, d, sl * 512:(sl + 1) * 512])
                        nc.sync.dma_start(zin_d.ap()[d, tb], stg[:])

            # ---- P2..P4 pool (reuses the xg region)
            p2pool = tc.alloc_tile_pool(name="p2", bufs=2)
            ohm_sb = p2pool.tile([T, NTOK], BF16, name="ohm_sb", bufs=1)
            nc.sync.dma_start(ohm_sb[:], ohm[:])
            vm_sb = p2pool.tile([T, NTOK], BF16, name="vm_sb", bufs=1)
            nc.sync.dma_start(vm_sb[:], vmask[:])

            # ---- P2: LSTM steps (fwd and bwd as separate chains)
            c_st = {}
            for d in range(2):
                c_st[d] = spool.tile([BC, HD], F32, tag=f"c{d}", name=f"c{d}")
                nc.sync.dma_start(c_st[d][:], c0.ap()[d * BC:(d + 1) * BC, :])

            zwin = {0: [None] * (S // 4), 1: [None] * (S // 4)}
            for t in range(S):
                ch = t // 4
                ro = t % 4
                for d in range(2):
                    if ro == 0:
                        zw = p2pool.tile([BC, 4, G4], BF16, tag=f"zw{d}", name=f"zw{d}")
                        nc.sync.dma_start(
                            zw[:],
                            zin_d.ap()[d, ch].rearrange("(s b) g -> b s g", s=4))
                        zwin[d][ch] = zw
                    zw = zwin[d][ch]

                    z_ps = zups.tile([BC, G4], F32, tag=f"zps{d}")
                    for sl in range(2):
                        gsl = slice(sl * 512, (sl + 1) * 512)
                        nc.tensor.matmul(
                            z_ps[:, gsl], ident_sb[0:BC, 0:32],
                            zw[:, ro, gsl], start=True, stop=False)
                        for k in range(2):
                            if t == 0:
                                hk = h0t_sb[:, k, d * BC:(d + 1) * BC]
                            elif d == 0:
                                hk = hts[0][:, k, (t - 1) * BC:t * BC]
                            else:
                                # bwd h_{t-1} lives at original pos S-1-(t-1)
                                hk = hts[1][:, k, (S - t) * BC:(S - t + 1) * BC]
                            nc.tensor.matmul(
                                z_ps[:, gsl], hk,
                                whh_sb[:, d, k, gsl],
                                start=False, stop=(k == 1))

                    cell = wpool.tile([BC, 1792], BF16, tag=f"cell{d}",
                                      name=f"cell{d}", bufs=3)
                    sig = cell[:, 0:768]
                    tg = cell[:, 768:G4]
                    t1 = cell[:, G4:G4 + HD]
                    th = cell[:, G4 + HD:G4 + 2 * HD]
                    h_sb = cell[:, G4 + 2 * HD:G4 + 3 * HD]
                    nc.scalar.activation(sig, z_ps[:, 0:768], AF.Sigmoid)
                    nc.scalar.activation(tg, z_ps[:, 768:G4], AF.Tanh)
                    nc.vector.tensor_mul(t1, sig[:, 0:HD], tg)
                    c_old = c_st[d]
                    c_st[d] = spool.tile([BC, HD], F32, tag=f"c{d}", name=f"c{d}")
                    nc.vector.tensor_mul(c_st[d][:], sig[:, HD:2 * HD], c_old[:])
                    nc.vector.tensor_add(c_st[d][:], c_st[d][:], t1)
                    nc.scalar.activation(th, c_st[d][:], AF.Tanh)
                    nc.vector.tensor_mul(h_sb, sig[:, 2 * HD:768], th)

                    # transpose h -> hT (feature-major) into the hT buffer
                    col = (t if d == 0 else S - 1 - t) * BC
                    for k in range(2):
                        tps = smps.tile([128, BC], BF16, tag="sm")
                        nc.tensor.transpose(
                            tps[:], h_sb[:, k * 128:(k + 1) * 128],
                            ident_sb[0:BC, 32:64])
                        if (d + k) % 2 == 0:
                            nc.scalar.copy(hts[d][:, k, col:col + BC], tps[:])
                        else:
                            nc.vector.tensor_copy(hts[d][:, k, col:col + BC],
                                                  tps[:])

            # ---- P3: emissions
            em_accs = []
            for tb in range(NTOK // 512):        # 8 blocks
                blk = slice(tb * 512, (tb + 1) * 512)
                ps = mmps.tile([T, 512], F32, tag="mm")
                for k in range(2):
                    nc.tensor.matmul(ps[:], wout_sb[:, k, :],
                                     hts[0][:, k, blk],
                                     start=(k == 0), stop=False)
                for k in range(2):
                    nc.tensor.matmul(ps[:], wout_sb[:, 2 + k, :],
                                     hts[1][:, k, blk],
                                     start=False, stop=(k == 1))
                acc = wpool.tile([T, 1], F32, tag="emacc" + str(tb), bufs=1, name=f"emacc{tb}")
                scr = wpool.tile([T, 512], F32, tag="ttrscr")
                nc.vector.tensor_mul(scr[:], ps[:], ohm_sb[:, blk])
                nc.vector.tensor_reduce(acc[:], scr[:],
                                        axis=mybir.AxisListType.X, op=ALU.add)
                em_accs.append(acc)
                nc.scalar.copy(em_sb[0:T, blk], ps[:])

            # exp(em + b_out) in place; first 32 cols also get start_trans
            bstart = wpool.tile([T, 1], F32, tag="bstart")
            nc.vector.tensor_add(bstart[:], tab_sb[:, 78:79], tab_sb[:, 76:77])
            nc.scalar.activation(em_sb[0:T, 0:BC], em_sb[0:T, 0:BC],
                                 AF.Exp, bias=bstart[:])
            nc.scalar.activation(em_sb[0:T, BC:512], em_sb[0:T, BC:512],
                                 AF.Exp, bias=tab_sb[:, 78:79])
            for tb in range(1, NTOK // 512):
                blk = slice(tb * 512, (tb + 1) * 512)
                nc.scalar.activation(em_sb[0:T, blk], em_sb[0:T, blk],
                                     AF.Exp, bias=tab_sb[:, 78:79])
            # zero padded positions (rows 0:76); absorber row from host
            for tb in range(NTOK // 512):
                blk = slice(tb * 512, (tb + 1) * 512)
                nc.vector.tensor_mul(em_sb[0:T, blk], em_sb[0:T, blk],
                                     vm_sb[:, blk])
            nc.sync.dma_start(em_sb[T:TA, :], padrow[:])

            # ---- P4: CRF forward in scaled linear space
            mp_sb = cpool.tile([TA, TA], F32)
            nc.scalar.activation(mp_sb[0:T, 0:T], tab_sb[:, 0:T], AF.Exp,
                                 bias=tab_sb[:, 79:80])
            nc.scalar.activation(mp_sb[0:T, T:TA], tab_sb[:, 77:78], AF.Exp,
                                 bias=tab_sb[:, 79:80])
            nc.sync.dma_start(mp_sb[T:TA, 0:TA], absrow.ap()[:, 0:TA])
            eend_sb = cpool.tile([TA, 1], F32)
            nc.scalar.activation(eend_sb[0:T, :], tab_sb[:, 77:78], AF.Exp)
            nc.sync.dma_start(eend_sb[T:TA, :], absrow.ap()[:, 77:78])

            a_prev = em_sb[0:TA, 0:BC]
            for t in range(1, S):
                aps = smps.tile([TA, BC], F32, tag="sm")
                nc.tensor.matmul(aps[:, 0:BC], mp_sb[:], a_prev,
                                 start=True, stop=True)
                a_new = spool.tile([TA, BC], F32, tag="a")
                nc.vector.tensor_mul(a_new[:], aps[:, 0:BC],
                                     em_sb[0:TA, t * BC:(t + 1) * BC])
                a_prev = a_new[:]

            sps = smps.tile([1, BC], F32, tag="sm")
            nc.tensor.matmul(sps[:, 0:BC], eend_sb[:], a_prev,
                             start=True, stop=True)
            logs = wpool.tile([1, BC], F32, tag="logs")
            nc.scalar.activation(logs[:], sps[:, 0:BC], AF.Ln)
            logsum = wpool.tile([1, 1], F32, tag="logsum")
            nc.vector.tensor_reduce(logsum[:], logs[:],
                                    axis=mybir.AxisListType.X, op=ALU.add)

            # gold score: table part
            gacc = wpool.tile([T, 1], F32, tag="gacc")
            scr2 = wpool.tile([T, 79], F32, tag="scr2")
            nc.vector.tensor_mul(scr2[:], gcnt_sb[:], tab_sb[:, 0:79])
            nc.vector.tensor_reduce(gacc[:], scr2[:],
                                    axis=mybir.AxisListType.X, op=ALU.add)
            tot = wpool.tile([T, 1], F32, tag="tot")
            nc.vector.tensor_add(tot[:], gacc[:], em_accs[0][:])
            for acc in em_accs[1:]:
                nc.vector.tensor_add(tot[:], tot[:], acc[:])
            ones = cpool.tile([T, 1], F32)
            nc.vector.memset(ones[:], 1.0)
            scps = smps.tile([1, 1], F32, tag="sm")
            nc.tensor.matmul(scps[:, 0:1], tot[:], ones[:],
                             start=True, stop=True)

            res = wpool.tile([1, 2], F32, tag="res")
            nc.vector.tensor_copy(res[:, 0:1], logsum[:])
            nc.vector.tensor_copy(res[:, 1:2], scps[:, 0:1])
            nc.sync.dma_start(out_d[:], res[:])
            p2pool.release()

    return nc


# ---------------------------------------------------------------- host side
def _gate_perm():
    """PyTorch gate order i,f,g,o -> reordered i,f,o,g (rows of W/b)."""
    return np.concatenate([
        np.arange(0, HD),            # i
        np.arange(HD, 2 * HD),       # f
        np.arange(3 * HD, 4 * HD),   # o
        np.arange(2 * HD, 3 * HD),   # g
    ])


def _pack_w_kxg(w, perm, nchunks):
    """w: [G4, kdim] -> [128, nchunks, G4] bf16, [p, c, g] = w[perm[g], c*128+p]."""
    wp = np.asarray(w)[perm, :]
    out = np.empty((128, nchunks, G4), dtype=ml_dtypes.bfloat16)
    for c in range(nchunks):
        out[:, c, :] = wp[:, c * 128:(c + 1) * 128].T.astype(ml_dtypes.bfloat16)
    return out


def _pack_idx(flat_ids):
    """flat token ids [NTOK] -> int16 [128, NTOK//16] wrap-16 layout."""
    out = np.zeros((128, NTOK // 16), dtype=np.int16)
    out[:16, :] = flat_ids.astype(np.int16).reshape(NTOK // 16, 16).T
    return out


def prep_inputs(inputs):
    """Build per-core input maps + host constants."""
    ids = np.asarray(inputs["input_ids"])
    tags = np.asarray(inputs["tag_ids"])
    lengths = np.asarray(inputs["lengths"])
    perm = _gate_perm()

    embed_bf = np.asarray(inputs["embed_table"]).astype(ml_dtypes.bfloat16)
    def gather_xt(flat_ids):
        g = embed_bf[flat_ids]                       # [NTOK, E] bf16
        return np.ascontiguousarray(
            g.reshape(NTOK, 4, 128).transpose(2, 1, 0))
    wih_pack = np.stack([_pack_w_kxg(inputs["W_ih_f"], perm, 4),
                         _pack_w_kxg(inputs["W_ih_b"], perm, 4)])
    whh_pack = np.stack([_pack_w_kxg(inputs["W_hh_f"], perm, 2),
                         _pack_w_kxg(inputs["W_hh_b"], perm, 2)])
    wo = np.asarray(inputs["W_out"])          # [T, H]
    wout_pack = np.empty((128, 4, T), dtype=ml_dtypes.bfloat16)
    for k in range(4):
        wout_pack[:, k, :] = wo[:, k * 128:(k + 1) * 128].T.astype(
            ml_dtypes.bfloat16)
    bias_f = (np.asarray(inputs["b_ih_f"]) + np.asarray(inputs["b_hh_f"]))[perm]
    bias_b = (np.asarray(inputs["b_ih_b"]) + np.asarray(inputs["b_hh_b"]))[perm]
    biasr = np.stack([np.broadcast_to(bias_f, (128, G4)),
                      np.broadcast_to(bias_b, (128, G4))]).astype(
                          ml_dtypes.bfloat16)

    ident = np.zeros((128, 96), dtype=ml_dtypes.bfloat16)
    for p in range(128):
        ident[p, p % 32] = 1
    for p in range(BC):
        ident[p, 32 + p] = 1

    trans = np.asarray(inputs["trans"]).astype(np.float64)
    kappa = float(np.log(np.exp(trans).sum(axis=0).mean()))
    tables = np.zeros((T, 80), dtype=np.float32)
    tables[:, 0:T] = trans.astype(np.float32)
    tables[:, 76] = np.asarray(inputs["start_trans"])
    tables[:, 77] = np.asarray(inputs["end_trans"])
    tables[:, 78] = np.asarray(inputs["b_out"])
    tables[:, 79] = -kappa

    h0 = np.asarray(inputs["h0"])             # [2, B, HD]
    c0 = np.asarray(inputs["c0"])

    in_maps = []
    k_len_total = 0
    for c in range(N_CORES):
        bs = slice(c * BC, (c + 1) * BC)
        ids_c = ids[bs]
        tags_c = tags[bs]
        len_c = lengths[bs].astype(np.int64)
        k_len_total += int(np.minimum(len_c, S - 1).sum())

        idx_f = ids_c.T.reshape(-1)                    # token (s, b) order
        idx_b = ids_c[:, ::-1].T.reshape(-1)
        xt = np.stack([gather_xt(idx_f), gather_xt(idx_b)])

        svec = np.arange(S)[None, :]
        valid = (svec < len_c[:, None]).T.reshape(-1)  # [(s, b)]
        ohm = np.zeros((T, NTOK), dtype=ml_dtypes.bfloat16)
        tt = tags_c.T.reshape(-1)
        pos = np.arange(NTOK)
        ohm[tt[valid], pos[valid]] = 1
        vm = np.broadcast_to(valid.astype(ml_dtypes.bfloat16),
                             (T, NTOK)).copy()
        padr = (~valid).astype(np.float32)[None, :]

        Cm = np.zeros((T, T), dtype=np.float32)
        h0v = np.zeros(T, dtype=np.float32)
        hLv = np.zeros(T, dtype=np.float32)
        for b in range(BC):
            L = int(len_c[b])
            tg = tags_c[b, :L]
            np.add.at(Cm, (tg[:-1], tg[1:]), 1)
            h0v[tg[0]] += 1
            hLv[tg[-1]] += 1
        nv = ohm.astype(np.float32).sum(axis=1)
        gcnt = np.concatenate([Cm, h0v[:, None], hLv[:, None], nv[:, None]],
                              axis=1)

        h0t = np.zeros((128, 2, 2 * BC), dtype=ml_dtypes.bfloat16)
        for k in range(2):
            h0t[:, k, 0:BC] = h0[0][bs][:, k * 128:(k + 1) * 128].T
            h0t[:, k, BC:2 * BC] = h0[1][bs][:, k * 128:(k + 1) * 128].T
        c0c = np.concatenate([c0[0][bs], c0[1][bs]], axis=0).astype(np.float32)

        absrow = np.zeros((1, 80), dtype=np.float32)
        absrow[0, 76] = 1.0
        absrow[0, 77] = 1.0
        in_maps.append(dict(
            xt=xt, wih=wih_pack, whh=whh_pack,
            wout=wout_pack, biasr=biasr, h0t=h0t, c0=c0c, ident=ident,
            tables=tables, gcnt=gcnt.astype(np.float32), ohm=ohm,
            vmask=vm, padrow=padr, absrow=absrow,
        ))

    return in_maps, dict(kappa=kappa, k_len_total=k_len_total)


def finalize(results, host):
    logz = sum(float(r["out"][0, 0]) for r in results)
    score = sum(float(r["out"][0, 1]) for r in results)
    logz += host["kappa"] * host["k_len_total"]
    return np.float32((logz - score) / B)


# ---------------------------------------------------------------- entry point
_COMPILED = {}


def kernel(**inputs):
    """Full-input BiLSTM-CRF loss on 8 NeuronCores (data parallel)."""
    from concourse.bass_utils import run_bass_kernel_spmd
    in_maps, host = prep_inputs(inputs)
    if "nc" not in _COMPILED:
        _COMPILED["nc"] = build_nc()
    nc = _COMPILED["nc"]
    res = run_bass_kernel_spmd(nc, in_maps, core_ids=list(range(N_CORES)))
    return np.asarray(finalize(res.results, host))



# revision 13
# speedup vs baseline: 1.8919x; 1.8919x over previous
"""BiLSTM-CRF loss kernel for Trainium2, 8-core data parallel.

Per-core (batch shard of 32), feature-major ("transposed") layout throughout:
gates/features live on partitions, batch on the free dim, so every elementwise
op runs at 128-partition occupancy with a small free size.

  - z_t for each direction accumulates in PSUM as [128 gates-in-chunk,
    8 chunks, batch]: per (chunk, dir) group = 1 bias matmul (K=1 ones rhs)
    + 4 x-projection matmuls (xg in [E, token] layout, consumed in-loop; no
    DRAM z roundtrip) + 2 recurrent matmuls off the transposed h buffer.
  - one sigmoid covers all 8 gate chunks; the g-gate rows of W/b are
    host-prescaled by 2 so tanh(g) = 2*sigmoid(z_g) - 1, done as a single
    DVE scalar_tensor_tensor. f*c runs on GpSimd (Pool) off the DVE path.
  - h = sigma_o * tanh(c) is written directly into the persistent transposed
    h buffer [128, k, dir, token] feeding both the next step's matmuls and
    the emission matmuls -- no PE transposes anywhere.
  - emissions (em = Wout.[hf;hb]) are computed per 512-token block as soon
    as both chains have covered it, with exp/vmask/gold-dot fused in.
  - CRF partition function in scaled linear space with an absorbing 77th
    tag: meet-in-the-middle (alpha forward 64 steps, beta/gamma backward 64
    steps, run concurrently), Z = alpha_63 . (M gamma_64).
Host combines the 8 per-core partial sums into the scalar loss.
"""

import numpy as np
import ml_dtypes

import concourse.bass as bass
import concourse.mybir as mybir
from concourse.tile import TileContext
from concourse.vector_clock import ScopedClock
from concourse.alu_op_type import AluOpType as ALU

N_CORES = 8
B, S, E, HD, T, V = 256, 128, 512, 256, 76, 30000
BC = B // N_CORES          # 32 batch per core
G4 = 4 * HD                # 1024 gates per direction
TA = T + 1                 # 77 tags with absorber
NTOK = S * BC              # 4096 tokens per direction per core

dt = mybir.dt
F32, BF16 = dt.float32, dt.bfloat16
AF = mybir.ActivationFunctionType

# ---------------------------------------------------------------- tile patch
# This walrus build rejects >1 sem wait on CTRL-class (Drain/NoOp)
# instructions; split the Tile tail-drain waits across preceding NOPs.
_MAX_WAITS = 1
_WAIT_LIMITS = {}


def _split_excess_waits(nc):
    """Non-DMA instructions accept only one sem wait on this walrus build;
    move excess waits onto NOPs spliced in front (same engine, same order)."""
    for f in nc.m.functions:
        stack = list(f.blocks)
        while stack:
            bb = stack.pop()
            for sub in getattr(bb, "blocks", []) or []:
                stack.append(sub)
            insts = getattr(bb, "instructions", None)
            if not insts:
                continue
            newlist = []
            changed = False
            for inst in insts:
                si = inst.sync_info
                lim = _WAIT_LIMITS.get(type(inst).__name__, 1)
                if si is not None and si.on_wait and len(si.on_wait) > lim:
                    waits = list(si.on_wait)
                    si.on_wait = waits[-lim:]
                    for w in waits[:-lim]:
                        nop = mybir.InstNoOp(
                            name=f"I-wsplit{nc.next_id()}", ins=[], outs=[],
                            engine=inst.engine,
                            sync_info=mybir.SyncInfo(on_wait=[w], on_update=[]),
                        )
                        newlist.append(nop)
                    changed = True
                newlist.append(inst)
            if changed:
                insts[:] = newlist


def _patched_drain_and_barrier(self, tick_clock, wait_clock):
    nc = self.nc
    _split_excess_waits(nc)
    nops = [nc.sync.nop(nofuse=True, hint=f"waitsplit{i}") for i in range(16)]
    drain_inst = nc.sync.drain()
    wait_clock.add_sem_waits(
        drain_inst.ins, ScopedClock({None: tick_clock.global_clock})
    )
    si = drain_inst.ins.sync_info
    if si is not None and si.on_wait and len(si.on_wait) > _MAX_WAITS:
        waits = list(si.on_wait)
        chunks = [waits[i:i + _MAX_WAITS] for i in range(0, len(waits), _MAX_WAITS)]
        si.on_wait = chunks[-1]
        assert len(chunks) - 1 <= len(nops), "too many wait chunks"
        for i, ch in enumerate(chunks[:-1]):
            ni = nops[i].ins
            if ni.sync_info is None:
                ni.sync_info = mybir.SyncInfo(on_wait=ch, on_update=[])
            else:
                ni.sync_info.on_wait = list(ni.sync_info.on_wait) + ch
    nc.all_engine_barrier()
    assert self.sems is not None
    popped = nc._tile_sem_poison_stack.pop()
    assert popped is self._sem_poison
    allsems = list(self.sems.allocated().values())
    for i in range(0, len(allsems), 8):
        nc.clear_and_free_semaphores(allsems[i:i + 8])
    nc.all_engine_barrier()


def apply_tile_patch():
    TileContext._drain_and_barrier = _patched_drain_and_barrier


# ---------------------------------------------------------------- builder
def build_nc():
    apply_tile_patch()
    nc = bass.Bass("TRN2", target_bir_lowering=False, debug=False,
                   num_devices=N_CORES)

    xt_d = nc.dram_tensor("xt", [2, 128, 4, NTOK], BF16, kind="ExternalInput")
    wih_d = nc.dram_tensor("wih", [128, 2, 4, G4], BF16, kind="ExternalInput")
    whh_d = nc.dram_tensor("whh", [128, 2, 2, G4], BF16, kind="ExternalInput")
    bias_d = nc.dram_tensor("biast", [1, 2, 8, 128], BF16, kind="ExternalInput")
    wout_d = nc.dram_tensor("wout", [128, 2, 2, T], BF16, kind="ExternalInput")
    h0t_d = nc.dram_tensor("h0t", [128, 2, 2, BC], BF16, kind="ExternalInput")
    c0t_d = nc.dram_tensor("c0t", [128, 2, 2, BC], F32, kind="ExternalInput")
    mp_d = nc.dram_tensor("mp", [TA, TA], BF16, kind="ExternalInput")
    mpt_d = nc.dram_tensor("mpt", [TA, TA], BF16, kind="ExternalInput")
    eend_d = nc.dram_tensor("eend", [TA, 1], F32, kind="ExternalInput")
    bvec_d = nc.dram_tensor("bvec", [T, 2], F32, kind="ExternalInput")
    ohm_d = nc.dram_tensor("ohm", [T, NTOK], BF16, kind="ExternalInput")
    vmask_d = nc.dram_tensor("vmask", [T, NTOK], BF16, kind="ExternalInput")
    padrow_d = nc.dram_tensor("padrow", [1, NTOK], BF16, kind="ExternalInput")
    out_d = nc.dram_tensor("out", [1, 2], F32, kind="ExternalOutput")

    NB = S // 16  # 8 emission blocks of 512 tokens
    # step (0-based) after which emission block b is fully available
    em_ready = {}
    for b in range(NB):
        r = max(16 * b + 15, S - 1 - 16 * b)
        em_ready.setdefault(r, []).append(b)

    with TileContext(nc) as tc:
        with (
            tc.tile_pool(name="const", bufs=1) as cpool,
            tc.tile_pool(name="hbuf", bufs=1) as hpool,
            tc.tile_pool(name="gate", bufs=3) as gpool,
            tc.tile_pool(name="cell", bufs=3) as spool,
            tc.tile_pool(name="work", bufs=3) as wpool,
            tc.tile_pool(name="zps", bufs=2, space="PSUM") as zps_pool,
            tc.tile_pool(name="emps", bufs=2, space="PSUM") as emps_pool,
            tc.tile_pool(name="crfps", bufs=2, space="PSUM") as crfps_pool,
        ):
            # ---- constants / weights into SBUF
            wih_sb = cpool.tile([128, 2, 4, G4], BF16)
            nc.sync.dma_start(wih_sb[:], wih_d[:])
            whh_sb = cpool.tile([128, 2, 2, G4], BF16)
            nc.sync.dma_start(whh_sb[:], whh_d[:])
            bias_sb = cpool.tile([1, 2, 8, 128], BF16)
            nc.sync.dma_start(bias_sb[:], bias_d[:])
            wout_sb = cpool.tile([128, 2, 2, T], BF16)
            nc.sync.dma_start(wout_sb[:], wout_d[:])
            h0t_sb = cpool.tile([128, 2, 2, BC], BF16)
            nc.sync.dma_start(h0t_sb[:], h0t_d[:])
            c0t_sb = cpool.tile([128, 2, 2, BC], F32)
            nc.sync.dma_start(c0t_sb[:], c0t_d[:])
            mp_sb = cpool.tile([TA, TA], BF16)
            nc.sync.dma_start(mp_sb[:], mp_d[:])
            mpt_sb = cpool.tile([TA, TA], BF16)
            nc.sync.dma_start(mpt_sb[:], mpt_d[:])
            eend_sb = cpool.tile([TA, 1], F32)
            nc.sync.dma_start(eend_sb[:], eend_d[:])
            bvec_sb = cpool.tile([T, 2], F32)
            nc.sync.dma_start(bvec_sb[:], bvec_d[:])

            ones1 = cpool.tile([1, BC], BF16)
            nc.vector.memset(ones1[:], 1.0)
            onesd = cpool.tile([128, 2, BC], F32)
            nc.vector.memset(onesd[:], 1.0)
            ones77 = cpool.tile([TA, 1], F32)
            nc.vector.memset(ones77[:], 1.0)

            # ---- big persistent buffers
            # xg: embeddings in [E-chunk, dir?, ...] layout; chunked DMA so
            # early steps start before the full 8MB lands.
            xg = {d: hpool.tile([128, 4, NTOK], BF16, name=f"xg{d}")
                  for d in range(2)}
            NCH = 4
            CW = NTOK // NCH
            for c in range(NCH):
                for d in range(2):
                    nc.sync.dma_start(
                        xg[d][:, :, c * CW:(c + 1) * CW],
                        xt_d.ap()[d, :, :, c * CW:(c + 1) * CW])
            # transposed h: [128, k-chunk, dir, token]
            hts = hpool.tile([128, 2, 2, NTOK], BF16, name="hts")
            # emissions (scaled-exp'd), bf16, absorber row 76
            em_sb = hpool.tile([TA, NTOK], BF16, name="em")
            ohm_sb = hpool.tile([T, NTOK], BF16, name="ohm")
            nc.sync.dma_start(ohm_sb[:], ohm_d[:])
            vm_sb = hpool.tile([T, NTOK], BF16, name="vm")
            nc.sync.dma_start(vm_sb[:], vmask_d[:])
            nc.sync.dma_start(em_sb[T:TA, :], padrow_d[:])

            # ---- z PSUM tile helpers -----------------------------------
            def emit_bias_x(zt, t):
                """bias + x-projection matmuls for step t into PSUM tile zt.
                zt layout: [128, 8 gate-chunk, 2*BC (dir, batch)]."""
                tok = slice(t * BC, (t + 1) * BC)
                for d in range(2):
                    dsl = slice(d * BC, (d + 1) * BC)
                    for gc in range(8):
                        nc.tensor.matmul(
                            zt[:, gc, dsl], bias_sb[:, d, gc, :],
                            ones1[:], start=True, stop=False)
                    for ek in range(4):
                        for gc in range(8):
                            nc.tensor.matmul(
                                zt[:, gc, dsl],
                                wih_sb[:, d, ek, gc * 128:(gc + 1) * 128],
                                xg[d][:, ek, tok], start=False, stop=False)

            def emit_h(zt, t):
                """recurrent matmuls (Whh . h_{t-1}) closing step t's groups."""
                for d in range(2):
                    dsl = slice(d * BC, (d + 1) * BC)
                    for k in range(2):
                        if t == 0:
                            hk = h0t_sb[:, d, k, :]
                        else:
                            col = (t - 1 if d == 0 else S - t) * BC
                            hk = hts[:, k, d, col:col + BC]
                        for gc in range(8):
                            nc.tensor.matmul(
                                zt[:, gc, dsl],
                                whh_sb[:, d, k, gc * 128:(gc + 1) * 128],
                                hk, start=False, stop=(k == 1))

            # ---- emission block -----------------------------------------
            em_accs = []

            def emit_emission(b):
                blk = slice(b * 512, (b + 1) * 512)
                ps = emps_pool.tile([T, 512], F32, tag="emps")
                i = 0
                for d in range(2):
                    for k in range(2):
                        nc.tensor.matmul(ps[:], wout_sb[:, k, d, :],
                                         hts[:, k, d, blk],
                                         start=(i == 0), stop=(i == 3))
                        i += 1
                # gold-path dot on raw em (b_out part handled on host)
                acc = wpool.tile([T, 1], F32, tag=f"emacc{b}", bufs=1,
                                 name=f"emacc{b}")
                scr = wpool.tile([T, 512], F32, tag="ttrscr")
                nc.vector.tensor_mul(scr[:], ps[:], ohm_sb[:, blk])
                nc.vector.tensor_reduce(acc[:], scr[:],
                                        axis=mybir.AxisListType.X, op=ALU.add)
                em_accs.append(acc)
                # scaled emissions: exp(em + b_out [+ start on col 0])
                if b == 0:
                    nc.scalar.activation(em_sb[0:T, 0:BC], ps[:, 0:BC],
                                         AF.Exp, bias=bvec_sb[:, 1:2])
                    nc.scalar.activation(em_sb[0:T, BC:512], ps[:, BC:512],
                                         AF.Exp, bias=bvec_sb[:, 0:1])
                else:
                    nc.scalar.activation(em_sb[0:T, blk], ps[:],
                                         AF.Exp, bias=bvec_sb[:, 0:1])
                nc.vector.tensor_mul(em_sb[0:T, blk], em_sb[0:T, blk],
                                     vm_sb[:, blk])

            # ---- LSTM loop ----------------------------------------------
            c_st = {d: c0t_sb[:, d, :, :] for d in range(2)}

            zt = zps_pool.tile([128, 8, 2 * BC], F32, tag="z")
            emit_bias_x(zt, 0)
            for t in range(S):
                emit_h(zt, t)
                zt_next = None
                if t < S - 1:
                    zt_next = zps_pool.tile([128, 8, 2 * BC], F32, tag="z")

                sig = {}
                for d in range(2):
                    dsl = slice(d * BC, (d + 1) * BC)
                    g = gpool.tile([128, 8, BC], F32, tag=f"g{d}",
                                   name=f"g{d}")
                    nc.scalar.activation(g[:], zt[:, :, dsl], AF.Sigmoid)
                    sig[d] = g
                    fc = spool.tile([128, 2, BC], F32, tag=f"fc{d}",
                                    name=f"fc{d}")
                    nc.vector.tensor_mul(fc[:], g[:, 2:4, :], c_st[d])
                    sig[d, "fc"] = fc
                cnew = {}
                for d in range(2):
                    g = sig[d]
                    tg = spool.tile([128, 2, BC], F32, tag=f"tg{d}",
                                    name=f"tg{d}")
                    # tanh(g) = 2*sigmoid(2g) - 1 (g-rows prescaled by 2)
                    nc.vector.scalar_tensor_tensor(
                        tg[:], g[:, 6:8, :], 2.0, onesd[:],
                        op0=ALU.mult, op1=ALU.subtract)
                    ig = spool.tile([128, 2, BC], F32, tag=f"ig{d}",
                                    name=f"ig{d}")
                    nc.vector.tensor_mul(ig[:], tg[:], g[:, 0:2, :])
                    cn = spool.tile([128, 2, BC], F32, tag=f"c{d}",
                                    name=f"c{d}")
                    nc.vector.tensor_add(cn[:], sig[d, "fc"][:], ig[:])
                    cnew[d] = cn
                th = {}
                for d in range(2):
                    thd = spool.tile([128, 2, BC], F32, tag=f"th{d}",
                                     name=f"th{d}")
                    nc.scalar.activation(thd[:], cnew[d][:], AF.Tanh)
                    th[d] = thd
                for d in range(2):
                    col = (t if d == 0 else S - 1 - t) * BC
                    nc.vector.tensor_mul(hts[:, :, d, col:col + BC],
                                         sig[d][:, 4:6, :], th[d][:])
                    c_st[d] = cnew[d][:]

                if zt_next is not None:
                    emit_bias_x(zt_next, t + 1)
                zt = zt_next

                for b in em_ready.get(t, []):
                    emit_emission(b)

            # ---- CRF: meet-in-the-middle forward/backward ---------------
            half = S // 2  # alpha covers em 0..63, gamma covers 127..64
            a_prev = em_sb[:, 0:BC]
            gma = gpool.tile([TA, BC], BF16, tag="gma", name="gma")
            nc.vector.tensor_scalar_mul(
                gma[:], em_sb[:, (S - 1) * BC:S * BC], eend_sb[:])
            g_prev = gma[:]
            for i in range(half - 1):
                ta_ = i + 1
                tb_ = S - 2 - i
                aps = crfps_pool.tile([TA, BC], F32, tag="crf")
                nc.tensor.matmul(aps[:], mp_sb[:], a_prev,
                                 start=True, stop=True)
                a_new = gpool.tile([TA, BC], BF16, tag="a", name="a")
                nc.vector.tensor_mul(
                    a_new[:], aps[:], em_sb[:, ta_ * BC:(ta_ + 1) * BC])
                a_prev = a_new[:]
                gps = crfps_pool.tile([TA, BC], F32, tag="crf")
                nc.tensor.matmul(gps[:], mpt_sb[:], g_prev,
                                 start=True, stop=True)
                g_new = gpool.tile([TA, BC], BF16, tag="gma", name="gma")
                nc.vector.tensor_mul(
                    g_new[:], gps[:], em_sb[:, tb_ * BC:(tb_ + 1) * BC])
                g_prev = g_new[:]

            # Z = alpha_63 . (M gamma_64)
            wps = crfps_pool.tile([TA, BC], F32, tag="crf")
            nc.tensor.matmul(wps[:], mpt_sb[:], g_prev, start=True, stop=True)
            u = wpool.tile([TA, BC], F32, tag="u")
            nc.vector.tensor_mul(u[:], wps[:], a_prev)
            zv = crfps_pool.tile([1, BC], F32, tag="zv", bufs=1)
            nc.tensor.matmul(zv[:], ones77[:], u[:], start=True, stop=True)
            logs = wpool.tile([1, BC], F32, tag="logs")
            nc.scalar.activation(logs[:], zv[:], AF.Ln)
            logsum = wpool.tile([1, 1], F32, tag="logsum")
            nc.vector.tensor_reduce(logsum[:], logs[:],
                                    axis=mybir.AxisListType.X, op=ALU.add)

            # ---- gold emission score sum --------------------------------
            tot = wpool.tile([T, 1], F32, tag="tot")
            nc.vector.tensor_add(tot[:], em_accs[0][:], em_accs[1][:])
            for acc in em_accs[2:]:
                nc.vector.tensor_add(tot[:], tot[:], acc[:])
            scps = crfps_pool.tile([1, 1], F32, tag="sc", bufs=1)
            nc.tensor.matmul(scps[:], ones77[0:T, :], tot[:],
                             start=True, stop=True)

            res = wpool.tile([1, 2], F32, tag="res")
            nc.vector.tensor_copy(res[:, 0:1], logsum[:])
            nc.vector.tensor_copy(res[:, 1:2], scps[:])
            nc.sync.dma_start(out_d[:], res[:])

    return nc


# ---------------------------------------------------------------- host side
def _gate_perm():
    """PyTorch gate order i,f,g,o -> reordered i,f,o,g (rows of W/b)."""
    return np.concatenate([
        np.arange(0, HD),            # i
        np.arange(HD, 2 * HD),       # f
        np.arange(3 * HD, 4 * HD),   # o
        np.arange(2 * HD, 3 * HD),   # g
    ])


def _pack_w_t(w, perm, nchunks, gscale):
    """w: [G4, kdim] -> [128, nchunks, G4] bf16 with
    out[p, c, g] = w[perm[g], c*128+p] * gscale[g]."""
    wp = np.asarray(w, dtype=np.float32)[perm, :] * gscale[:, None]
    out = np.empty((128, nchunks, G4), dtype=ml_dtypes.bfloat16)
    for c in range(nchunks):
        out[:, c, :] = wp[:, c * 128:(c + 1) * 128].T.astype(ml_dtypes.bfloat16)
    return out


def prep_inputs(inputs):
    """Build per-core input maps + host constants."""
    ids = np.asarray(inputs["input_ids"])
    tags = np.asarray(inputs["tag_ids"])
    lengths = np.asarray(inputs["lengths"])
    perm = _gate_perm()
    # gate g (index 768:1024 after perm) prescaled by 2 for the
    # tanh(x) = 2*sigmoid(2x)-1 identity
    gscale = np.ones(G4, dtype=np.float32)
    gscale[3 * HD:] = 2.0

    embed_bf = np.asarray(inputs["embed_table"]).astype(ml_dtypes.bfloat16)

    def gather_xt(flat_ids):
        g = embed_bf[flat_ids]                       # [NTOK, E] bf16
        return np.ascontiguousarray(
            g.reshape(NTOK, 4, 128).transpose(2, 1, 0))

    wih_pack = np.stack([_pack_w_t(inputs["W_ih_f"], perm, 4, gscale),
                         _pack_w_t(inputs["W_ih_b"], perm, 4, gscale)],
                        axis=1)                      # [128, 2, 4, G4]
    whh_pack = np.stack([_pack_w_t(inputs["W_hh_f"], perm, 2, gscale),
                         _pack_w_t(inputs["W_hh_b"], perm, 2, gscale)],
                        axis=1)                      # [128, 2, 2, G4]
    bias_f = ((np.asarray(inputs["b_ih_f"]) + np.asarray(inputs["b_hh_f"]))
              [perm] * gscale)
    bias_b = ((np.asarray(inputs["b_ih_b"]) + np.asarray(inputs["b_hh_b"]))
              [perm] * gscale)
    bias_pack = np.stack([bias_f.reshape(8, 128), bias_b.reshape(8, 128)]
                         )[None].astype(ml_dtypes.bfloat16)  # [1, 2, 8, 128]

    wo = np.asarray(inputs["W_out"])                 # [T, H]
    wout_pack = np.empty((128, 2, 2, T), dtype=ml_dtypes.bfloat16)
    for d in range(2):
        for k in range(2):
            sl = slice(d * 256 + k * 128, d * 256 + (k + 1) * 128)
            wout_pack[:, k, d, :] = wo[:, sl].T.astype(ml_dtypes.bfloat16)

    trans = np.asarray(inputs["trans"]).astype(np.float64)
    start_t = np.asarray(inputs["start_trans"]).astype(np.float64)
    end_t = np.asarray(inputs["end_trans"]).astype(np.float64)
    bout = np.asarray(inputs["b_out"]).astype(np.float64)
    kappa = float(np.log(np.exp(trans).sum(axis=0).mean()))

    mp = np.zeros((TA, TA), dtype=np.float64)
    mp[0:T, 0:T] = np.exp(trans - kappa)
    mp[0:T, T] = np.exp(end_t - kappa)
    mp[T, T] = 1.0
    eend = np.zeros((TA, 1), dtype=np.float32)
    eend[0:T, 0] = np.exp(end_t)
    eend[T, 0] = 1.0
    bvec = np.zeros((T, 2), dtype=np.float32)
    bvec[:, 0] = bout
    bvec[:, 1] = bout + start_t

    h0 = np.asarray(inputs["h0"])                    # [2, B, HD]
    c0 = np.asarray(inputs["c0"])

    in_maps = []
    k_len_total = 0
    gold_host_total = 0.0
    for c in range(N_CORES):
        bs = slice(c * BC, (c + 1) * BC)
        ids_c = ids[bs]
        tags_c = tags[bs]
        len_c = lengths[bs].astype(np.int64)
        k_len_total += int(np.minimum(len_c, S - 1).sum())

        idx_f = ids_c.T.reshape(-1)                    # token (s, b) order
        idx_b = ids_c[:, ::-1].T.reshape(-1)
        xt = np.stack([gather_xt(idx_f), gather_xt(idx_b)])

        svec = np.arange(S)[None, :]
        valid = (svec < len_c[:, None]).T.reshape(-1)  # [(s, b)]
        ohm = np.zeros((T, NTOK), dtype=ml_dtypes.bfloat16)
        tt = tags_c.T.reshape(-1)
        pos = np.arange(NTOK)
        ohm[tt[valid], pos[valid]] = 1
        vm = np.broadcast_to(valid.astype(ml_dtypes.bfloat16),
                             (T, NTOK)).copy()
        padr = (~valid).astype(ml_dtypes.bfloat16)[None, :]

        # gold-path table part (trans/start/end/b_out counts) on host
        gh = 0.0
        for b in range(BC):
            L = int(len_c[b])
            tg = tags_c[b, :L]
            gh += float(trans[tg[:-1], tg[1:]].sum())
            gh += float(start_t[tg[0]] + end_t[tg[-1]])
            gh += float(bout[tg].sum())
        gold_host_total += gh

        h0t = np.zeros((128, 2, 2, BC), dtype=ml_dtypes.bfloat16)
        c0t = np.zeros((128, 2, 2, BC), dtype=np.float32)
        for d in range(2):
            for k in range(2):
                h0t[:, d, k, :] = h0[d][bs][:, k * 128:(k + 1) * 128].T
                c0t[:, d, k, :] = c0[d][bs][:, k * 128:(k + 1) * 128].T

        in_maps.append(dict(
            xt=xt, wih=wih_pack, whh=whh_pack, biast=bias_pack,
            wout=wout_pack, h0t=h0t, c0t=c0t,
            mp=mp.astype(ml_dtypes.bfloat16),
            mpt=mp.T.copy().astype(ml_dtypes.bfloat16),
            eend=eend, bvec=bvec, ohm=ohm, vmask=vm, padrow=padr,
        ))

    return in_maps, dict(kappa=kappa, k_len_total=k_len_total,
                         gold_host_total=gold_host_total)


def finalize(results, host):
    logz = sum(float(r["out"][0, 0]) for r in results)
    gold_em = sum(float(r["out"][0, 1]) for r in results)
    logz += host["kappa"] * host["k_len_total"]
    score = gold_em + host["gold_host_total"]
    return np.float32((logz - score) / B)


# ---------------------------------------------------------------- entry point
_COMPILED = {}


def kernel(**inputs):
    """Full-input BiLSTM-CRF loss on 8 NeuronCores (data parallel)."""
    from concourse.bass_utils import run_bass_kernel_spmd
    in_maps, host = prep_inputs(inputs)
    if "nc" not in _COMPILED:
        _COMPILED["nc"] = build_nc()
    nc = _COMPILED["nc"]
    res = run_bass_kernel_spmd(nc, in_maps, core_ids=list(range(N_CORES)))
    return np.asarray(finalize(res.results, host))


# revision 21
# speedup vs baseline: 2.1043x; 1.1122x over previous
"""BiLSTM-CRF loss kernel for Trainium2, 8-core data parallel.

Per-core (batch shard of 32), feature-major ("transposed") layout throughout:
gates/features live on partitions, batch on the free dim, so every elementwise
op runs at 128-partition occupancy with a small free size.

  - z_t for each direction accumulates in PSUM as [128 gates-in-chunk,
    8 chunks, batch]: per (chunk, dir) group = 1 bias matmul (K=1 ones rhs)
    + 4 x-projection matmuls (xg in [E, token] layout, consumed in-loop; no
    DRAM z roundtrip) + 2 recurrent matmuls off the transposed h buffer.
  - one sigmoid covers all 8 gate chunks; the g-gate rows of W/b are
    host-prescaled by 2 so tanh(g) = 2*sigmoid(z_g) - 1, done as a single
    DVE scalar_tensor_tensor. f*c runs on GpSimd (Pool) off the DVE path.
  - h = sigma_o * tanh(c) is written directly into the persistent transposed
    h buffer [128, k, dir, token] feeding both the next step's matmuls and
    the emission matmuls -- no PE transposes anywhere.
  - emissions (em = Wout.[hf;hb]) are computed per 512-token block as soon
    as both chains have covered it, with exp/vmask/gold-dot fused in.
  - CRF partition function in scaled linear space with an absorbing 77th
    tag: meet-in-the-middle (alpha forward 64 steps, beta/gamma backward 64
    steps, run concurrently), Z = alpha_63 . (M gamma_64).
Host combines the 8 per-core partial sums into the scalar loss.
"""

import numpy as np
import ml_dtypes

import concourse.bass as bass
import concourse.mybir as mybir
from concourse.tile import TileContext
from concourse.vector_clock import ScopedClock
from concourse.alu_op_type import AluOpType as ALU

N_CORES = 8
B, S, E, HD, T, V = 256, 128, 512, 256, 76, 30000
BC = B // N_CORES          # 32 batch per core
G4 = 4 * HD                # 1024 gates per direction
TA = T + 1                 # 77 tags with absorber
NTOK = S * BC              # 4096 tokens per direction per core

dt = mybir.dt
F32, BF16 = dt.float32, dt.bfloat16
AF = mybir.ActivationFunctionType

# ---------------------------------------------------------------- tile patch
# This walrus build rejects >1 sem wait on CTRL-class (Drain/NoOp)
# instructions; split the Tile tail-drain waits across preceding NOPs.
_MAX_WAITS = 1
_WAIT_LIMITS = {}


def _split_excess_waits(nc):
    """Non-DMA instructions accept only one sem wait on this walrus build;
    move excess waits onto NOPs spliced in front (same engine, same order)."""
    for f in nc.m.functions:
        stack = list(f.blocks)
        while stack:
            bb = stack.pop()
            for sub in getattr(bb, "blocks", []) or []:
                stack.append(sub)
            insts = getattr(bb, "instructions", None)
            if not insts:
                continue
            newlist = []
            changed = False
            for inst in insts:
                si = inst.sync_info
                lim = _WAIT_LIMITS.get(type(inst).__name__, 1)
                if si is not None and si.on_wait and len(si.on_wait) > lim:
                    waits = list(si.on_wait)
                    si.on_wait = waits[-lim:]
                    for w in waits[:-lim]:
                        nop = mybir.InstNoOp(
                            name=f"I-wsplit{nc.next_id()}", ins=[], outs=[],
                            engine=inst.engine,
                            sync_info=mybir.SyncInfo(on_wait=[w], on_update=[]),
                        )
                        newlist.append(nop)
                    changed = True
                newlist.append(inst)
            if changed:
                insts[:] = newlist


def _patched_drain_and_barrier(self, tick_clock, wait_clock):
    nc = self.nc
    _split_excess_waits(nc)
    nops = [nc.sync.nop(nofuse=True, hint=f"waitsplit{i}") for i in range(16)]
    drain_inst = nc.sync.drain()
    wait_clock.add_sem_waits(
        drain_inst.ins, ScopedClock({None: tick_clock.global_clock})
    )
    si = drain_inst.ins.sync_info
    if si is not None and si.on_wait and len(si.on_wait) > _MAX_WAITS:
        waits = list(si.on_wait)
        chunks = [waits[i:i + _MAX_WAITS] for i in range(0, len(waits), _MAX_WAITS)]
        si.on_wait = chunks[-1]
        assert len(chunks) - 1 <= len(nops), "too many wait chunks"
        for i, ch in enumerate(chunks[:-1]):
            ni = nops[i].ins
            if ni.sync_info is None:
                ni.sync_info = mybir.SyncInfo(on_wait=ch, on_update=[])
            else:
                ni.sync_info.on_wait = list(ni.sync_info.on_wait) + ch
    nc.all_engine_barrier()
    assert self.sems is not None
    popped = nc._tile_sem_poison_stack.pop()
    assert popped is self._sem_poison
    allsems = list(self.sems.allocated().values())
    for i in range(0, len(allsems), 8):
        nc.clear_and_free_semaphores(allsems[i:i + 8])
    nc.all_engine_barrier()


def apply_tile_patch():
    TileContext._drain_and_barrier = _patched_drain_and_barrier


# ---------------------------------------------------------------- builder
def build_nc():
    apply_tile_patch()
    nc = bass.Bass("TRN2", target_bir_lowering=False, debug=False,
                   num_devices=N_CORES)

    xt_d = nc.dram_tensor("xt", [2, 128, 4, NTOK], BF16, kind="ExternalInput")
    wih_d = nc.dram_tensor("wih", [128, 2, 4, G4], BF16, kind="ExternalInput")
    whh_d = nc.dram_tensor("whh", [128, 2, 2, G4], BF16, kind="ExternalInput")
    bias_d = nc.dram_tensor("biast", [1, 2, 8, 128], BF16, kind="ExternalInput")
    wout_d = nc.dram_tensor("wout", [128, 2, 2, T], BF16, kind="ExternalInput")
    h0t_d = nc.dram_tensor("h0t", [128, 2, 2, BC], BF16, kind="ExternalInput")
    c0t_d = nc.dram_tensor("c0t", [128, 2, 2, BC], F32, kind="ExternalInput")
    mp_d = nc.dram_tensor("mp", [TA, TA], BF16, kind="ExternalInput")
    mpt_d = nc.dram_tensor("mpt", [TA, TA], BF16, kind="ExternalInput")
    eend_d = nc.dram_tensor("eend", [TA, 1], F32, kind="ExternalInput")
    bvec_d = nc.dram_tensor("bvec", [T, 2], F32, kind="ExternalInput")
    ohm_d = nc.dram_tensor("ohm", [T, NTOK], BF16, kind="ExternalInput")
    vmask_d = nc.dram_tensor("vmask", [T, NTOK], BF16, kind="ExternalInput")
    padrow_d = nc.dram_tensor("padrow", [1, NTOK], BF16, kind="ExternalInput")
    out_d = nc.dram_tensor("out", [1, 2], F32, kind="ExternalOutput")

    NB = S // 16  # 8 emission blocks of 512 tokens
    # step (0-based) after which emission block b is fully available
    em_ready = {}
    for b in range(NB):
        r = max(16 * b + 15, S - 1 - 16 * b)
        em_ready.setdefault(r, []).append(b)

    with TileContext(nc) as tc:
        with (
            tc.tile_pool(name="const", bufs=1) as cpool,
            tc.tile_pool(name="hbuf", bufs=1) as hpool,
            tc.tile_pool(name="gate", bufs=3) as gpool,
            tc.tile_pool(name="cell", bufs=3) as spool,
            tc.tile_pool(name="work", bufs=3) as wpool,
            tc.tile_pool(name="zps", bufs=2, space="PSUM") as zps_pool,
            tc.tile_pool(name="emps", bufs=1, space="PSUM") as emps_pool,
            tc.tile_pool(name="crfps", bufs=2, space="PSUM") as crfps_pool,
        ):
            # ---- constants / weights into SBUF
            wih_sb = cpool.tile([128, 2, 4, G4], BF16)
            nc.sync.dma_start(wih_sb[:], wih_d[:])
            whh_sb = cpool.tile([128, 2, 2, G4], BF16)
            nc.sync.dma_start(whh_sb[:], whh_d[:])
            bias_sb = cpool.tile([1, 2, 8, 128], BF16)
            nc.sync.dma_start(bias_sb[:], bias_d[:])
            wout_sb = cpool.tile([128, 2, 2, T], BF16)
            nc.sync.dma_start(wout_sb[:], wout_d[:])
            h0t_sb = cpool.tile([128, 2, 2, BC], BF16)
            nc.sync.dma_start(h0t_sb[:], h0t_d[:])
            c0t_sb = cpool.tile([128, 2, 2, BC], F32)
            nc.sync.dma_start(c0t_sb[:], c0t_d[:])
            mp_sb = cpool.tile([TA, TA], BF16)
            nc.sync.dma_start(mp_sb[:], mp_d[:])
            mpt_sb = cpool.tile([TA, TA], BF16)
            nc.sync.dma_start(mpt_sb[:], mpt_d[:])
            eend_sb = cpool.tile([TA, 1], F32)
            nc.sync.dma_start(eend_sb[:], eend_d[:])
            bvec_sb = cpool.tile([T, 2], F32)
            nc.sync.dma_start(bvec_sb[:], bvec_d[:])

            ones1 = cpool.tile([1, BC], BF16)
            nc.vector.memset(ones1[:], 1.0)
            onesd = cpool.tile([128, 2, BC], F32)
            nc.vector.memset(onesd[:], 1.0)
            ones77 = cpool.tile([TA, 1], F32)
            nc.vector.memset(ones77[:], 1.0)

            # ---- big persistent buffers
            # xg: embeddings in [E-chunk, dir?, ...] layout; chunked DMA so
            # early steps start before the full 8MB lands.
            xg = {d: hpool.tile([128, 4, NTOK], BF16, name=f"xg{d}")
                  for d in range(2)}
            NCH = 4
            CW = NTOK // NCH
            for c in range(NCH):
                for d in range(2):
                    nc.sync.dma_start(
                        xg[d][:, :, c * CW:(c + 1) * CW],
                        xt_d.ap()[d, :, :, c * CW:(c + 1) * CW])
            # transposed h, one tile per direction: [128, k-chunk, token]
            hts = {d: hpool.tile([128, 2, NTOK], BF16, name=f"hts{d}")
                   for d in range(2)}
            # emissions (scaled-exp'd), bf16, absorber row 76
            em_sb = hpool.tile([TA, NTOK], BF16, name="em")
            ohm_sb = hpool.tile([T, NTOK], BF16, name="ohm")
            nc.sync.dma_start(ohm_sb[:], ohm_d[:])
            vm_sb = hpool.tile([T, NTOK], BF16, name="vm")
            nc.sync.dma_start(vm_sb[:], vmask_d[:])
            nc.sync.dma_start(em_sb[T:TA, :], padrow_d[:])

            # ---- z PSUM tile helpers -----------------------------------
            def emit_bias_x(zt, d, t):
                """bias + x-projection matmuls of direction d for step t into
                PSUM tile zt [128, 8 gate-chunk, BC]."""
                tok = slice(t * BC, (t + 1) * BC)
                for gc in range(8):
                    nc.tensor.matmul(
                        zt[:, gc, :], bias_sb[:, d, gc, :],
                        ones1[:], start=True, stop=False)
                for ek in range(4):
                    for gc in range(8):
                        nc.tensor.matmul(
                            zt[:, gc, :],
                            wih_sb[:, d, ek, gc * 128:(gc + 1) * 128],
                            xg[d][:, ek, tok], start=False, stop=False)

            def emit_h(zt, d, t):
                """recurrent matmuls (Whh . h_{t-1}) closing step t's groups."""
                for k in range(2):
                    if t == 0:
                        hk = h0t_sb[:, d, k, :]
                    else:
                        col = (t - 1 if d == 0 else S - t) * BC
                        hk = hts[d][:, k, col:col + BC]
                    for gc in range(8):
                        nc.tensor.matmul(
                            zt[:, gc, :],
                            whh_sb[:, d, k, gc * 128:(gc + 1) * 128],
                            hk, start=False, stop=(k == 1))

            # ---- emission block -----------------------------------------
            em_accs = []

            def emit_emission(b):
                blk = slice(b * 512, (b + 1) * 512)
                ps = emps_pool.tile([T, 512], F32, tag="emps")
                i = 0
                for d in range(2):
                    for k in range(2):
                        nc.tensor.matmul(ps[:], wout_sb[:, k, d, :],
                                         hts[d][:, k, blk],
                                         start=(i == 0), stop=(i == 3))
                        i += 1
                # gold-path dot on raw em (b_out part handled on host)
                acc = wpool.tile([T, 1], F32, tag=f"emacc{b}", bufs=1,
                                 name=f"emacc{b}")
                scr = wpool.tile([T, 512], F32, tag="ttrscr")
                nc.vector.tensor_mul(scr[:], ps[:], ohm_sb[:, blk])
                nc.vector.tensor_reduce(acc[:], scr[:],
                                        axis=mybir.AxisListType.X, op=ALU.add)
                em_accs.append(acc)
                # scaled emissions: exp(em + b_out [+ start on col 0])
                if b == 0:
                    nc.scalar.activation(em_sb[0:T, 0:BC], ps[:, 0:BC],
                                         AF.Exp, bias=bvec_sb[:, 1:2])
                    nc.scalar.activation(em_sb[0:T, BC:512], ps[:, BC:512],
                                         AF.Exp, bias=bvec_sb[:, 0:1])
                else:
                    nc.scalar.activation(em_sb[0:T, blk], ps[:],
                                         AF.Exp, bias=bvec_sb[:, 0:1])
                nc.vector.tensor_mul(em_sb[0:T, blk], em_sb[0:T, blk],
                                     vm_sb[:, blk])

            # ---- LSTM loop ----------------------------------------------
            c_st = {d: c0t_sb[:, d, :, :] for d in range(2)}

            zt = {d: zps_pool.tile([128, 8, BC], F32, tag=f"z{d}",
                                         name=f"z{d}")
                  for d in range(2)}
            for d in range(2):
                emit_bias_x(zt[d], d, 0)
            for t in range(S):
                for d in range(2):
                    emit_h(zt[d], d, t)
                zt_next = None
                if t < S - 1:
                    zt_next = {
                        d: zps_pool.tile([128, 8, BC], F32, tag=f"z{d}",
                                         name=f"z{d}")
                        for d in range(2)}

                # Act: sig_f, sig_b, tanh_f, tanh_b (in-order);
                # DVE: full f cell chain first, then b, then the h muls.
                sig = {}
                for d in range(2):
                    g = gpool.tile([128, 8, BC], F32, tag=f"g{d}",
                                   name=f"g{d}")
                    nc.scalar.activation(g[:], zt[d][:], AF.Sigmoid)
                    sig[d] = g
                cnew = {}
                for d in range(2):
                    g = sig[d]
                    fc = spool.tile([128, 2, BC], F32, tag=f"fc{d}",
                                    name=f"fc{d}")
                    nc.vector.tensor_mul(fc[:], g[:, 2:4, :], c_st[d])
                    tg = spool.tile([128, 2, BC], F32, tag=f"tg{d}",
                                    name=f"tg{d}")
                    # tanh(g) = 2*sigmoid(2g) - 1 (g-rows prescaled by 2)
                    nc.vector.scalar_tensor_tensor(
                        tg[:], g[:, 6:8, :], 2.0, onesd[:],
                        op0=ALU.mult, op1=ALU.subtract)
                    ig = spool.tile([128, 2, BC], F32, tag=f"ig{d}",
                                    name=f"ig{d}")
                    nc.vector.tensor_mul(ig[:], tg[:], g[:, 0:2, :])
                    cn = spool.tile([128, 2, BC], F32, tag=f"c{d}",
                                    name=f"c{d}")
                    nc.vector.tensor_add(cn[:], fc[:], ig[:])
                    cnew[d] = cn
                th = {}
                for d in range(2):
                    thd = spool.tile([128, 2, BC], F32, tag=f"th{d}",
                                     name=f"th{d}")
                    nc.scalar.activation(thd[:], cnew[d][:], AF.Tanh)
                    th[d] = thd
                for d in range(2):
                    col = (t if d == 0 else S - 1 - t) * BC
                    nc.vector.tensor_mul(hts[d][:, :, col:col + BC],
                                         sig[d][:, 4:6, :], th[d][:])
                    c_st[d] = cnew[d][:]

                if zt_next is not None:
                    for d in range(2):
                        emit_bias_x(zt_next[d], d, t + 1)
                zt = zt_next

                for b in em_ready.get(t, []):
                    emit_emission(b)

            # ---- CRF: meet-in-the-middle forward/backward ---------------
            half = S // 2  # alpha covers em 0..63, gamma covers 127..64
            a_prev = em_sb[:, 0:BC]
            gma = gpool.tile([TA, BC], BF16, tag="gma", name="gma")
            nc.vector.tensor_scalar_mul(
                gma[:], em_sb[:, (S - 1) * BC:S * BC], eend_sb[:])
            g_prev = gma[:]
            for i in range(half - 1):
                ta_ = i + 1
                tb_ = S - 2 - i
                aps = crfps_pool.tile([TA, BC], F32, tag="crf")
                nc.tensor.matmul(aps[:], mp_sb[:], a_prev,
                                 start=True, stop=True)
                a_new = gpool.tile([TA, BC], BF16, tag="a", name="a")
                nc.vector.tensor_mul(
                    a_new[:], aps[:], em_sb[:, ta_ * BC:(ta_ + 1) * BC])
                a_prev = a_new[:]
                gps = crfps_pool.tile([TA, BC], F32, tag="crf")
                nc.tensor.matmul(gps[:], mpt_sb[:], g_prev,
                                 start=True, stop=True)
                g_new = gpool.tile([TA, BC], BF16, tag="gma", name="gma")
                nc.vector.tensor_mul(
                    g_new[:], gps[:], em_sb[:, tb_ * BC:(tb_ + 1) * BC])
                g_prev = g_new[:]

            # Z = alpha_63 . (M gamma_64)
            wps = crfps_pool.tile([TA, BC], F32, tag="crf")
            nc.tensor.matmul(wps[:], mpt_sb[:], g_prev, start=True, stop=True)
            u = wpool.tile([TA, BC], F32, tag="u")
            nc.vector.tensor_mul(u[:], wps[:], a_prev)
            zsc = crfps_pool.tile([1, BC + 8], F32, tag="zsc", bufs=1)
            nc.tensor.matmul(zsc[:, 0:BC], ones77[:], u[:],
                             start=True, stop=True)
            logs = wpool.tile([1, BC], F32, tag="logs")
            nc.scalar.activation(logs[:], zsc[:, 0:BC], AF.Ln)
            logsum = wpool.tile([1, 1], F32, tag="logsum")
            nc.vector.tensor_reduce(logsum[:], logs[:],
                                    axis=mybir.AxisListType.X, op=ALU.add)

            # ---- gold emission score sum --------------------------------
            tot = wpool.tile([T, 1], F32, tag="tot")
            nc.vector.tensor_add(tot[:], em_accs[0][:], em_accs[1][:])
            for acc in em_accs[2:]:
                nc.vector.tensor_add(tot[:], tot[:], acc[:])
            nc.tensor.matmul(zsc[:, BC:BC + 1], ones77[0:T, :], tot[:],
                             start=True, stop=True)

            res = wpool.tile([1, 2], F32, tag="res")
            nc.vector.tensor_copy(res[:, 0:1], logsum[:])
            nc.vector.tensor_copy(res[:, 1:2], zsc[:, BC:BC + 1])
            nc.sync.dma_start(out_d[:], res[:])

    return nc


# ---------------------------------------------------------------- host side
def _gate_perm():
    """PyTorch gate order i,f,g,o -> reordered i,f,o,g (rows of W/b)."""
    return np.concatenate([
        np.arange(0, HD),            # i
        np.arange(HD, 2 * HD),       # f
        np.arange(3 * HD, 4 * HD),   # o
        np.arange(2 * HD, 3 * HD),   # g
    ])


def _pack_w_t(w, perm, nchunks, gscale):
    """w: [G4, kdim] -> [128, nchunks, G4] bf16 with
    out[p, c, g] = w[perm[g], c*128+p] * gscale[g]."""
    wp = np.asarray(w, dtype=np.float32)[perm, :] * gscale[:, None]
    out = np.empty((128, nchunks, G4), dtype=ml_dtypes.bfloat16)
    for c in range(nchunks):
        out[:, c, :] = wp[:, c * 128:(c + 1) * 128].T.astype(ml_dtypes.bfloat16)
    return out


def prep_inputs(inputs):
    """Build per-core input maps + host constants."""
    ids = np.asarray(inputs["input_ids"])
    tags = np.asarray(inputs["tag_ids"])
    lengths = np.asarray(inputs["lengths"])
    perm = _gate_perm()
    # gate g (index 768:1024 after perm) prescaled by 2 for the
    # tanh(x) = 2*sigmoid(2x)-1 identity
    gscale = np.ones(G4, dtype=np.float32)
    gscale[3 * HD:] = 2.0

    embed_bf = np.asarray(inputs["embed_table"]).astype(ml_dtypes.bfloat16)

    def gather_xt(flat_ids):
        g = embed_bf[flat_ids]                       # [NTOK, E] bf16
        return np.ascontiguousarray(
            g.reshape(NTOK, 4, 128).transpose(2, 1, 0))

    wih_pack = np.stack([_pack_w_t(inputs["W_ih_f"], perm, 4, gscale),
                         _pack_w_t(inputs["W_ih_b"], perm, 4, gscale)],
                        axis=1)                      # [128, 2, 4, G4]
    whh_pack = np.stack([_pack_w_t(inputs["W_hh_f"], perm, 2, gscale),
                         _pack_w_t(inputs["W_hh_b"], perm, 2, gscale)],
                        axis=1)                      # [128, 2, 2, G4]
    bias_f = ((np.asarray(inputs["b_ih_f"]) + np.asarray(inputs["b_hh_f"]))
              [perm] * gscale)
    bias_b = ((np.asarray(inputs["b_ih_b"]) + np.asarray(inputs["b_hh_b"]))
              [perm] * gscale)
    bias_pack = np.stack([bias_f.reshape(8, 128), bias_b.reshape(8, 128)]
                         )[None].astype(ml_dtypes.bfloat16)  # [1, 2, 8, 128]

    wo = np.asarray(inputs["W_out"])                 # [T, H]
    wout_pack = np.empty((128, 2, 2, T), dtype=ml_dtypes.bfloat16)
    for d in range(2):
        for k in range(2):
            sl = slice(d * 256 + k * 128, d * 256 + (k + 1) * 128)
            wout_pack[:, k, d, :] = wo[:, sl].T.astype(ml_dtypes.bfloat16)

    trans = np.asarray(inputs["trans"]).astype(np.float64)
    start_t = np.asarray(inputs["start_trans"]).astype(np.float64)
    end_t = np.asarray(inputs["end_trans"]).astype(np.float64)
    bout = np.asarray(inputs["b_out"]).astype(np.float64)
    kappa = float(np.log(np.exp(trans).sum(axis=0).mean()))

    mp = np.zeros((TA, TA), dtype=np.float64)
    mp[0:T, 0:T] = np.exp(trans - kappa)
    mp[0:T, T] = np.exp(end_t - kappa)
    mp[T, T] = 1.0
    eend = np.zeros((TA, 1), dtype=np.float32)
    eend[0:T, 0] = np.exp(end_t)
    eend[T, 0] = 1.0
    bvec = np.zeros((T, 2), dtype=np.float32)
    bvec[:, 0] = bout
    bvec[:, 1] = bout + start_t

    h0 = np.asarray(inputs["h0"])                    # [2, B, HD]
    c0 = np.asarray(inputs["c0"])

    in_maps = []
    k_len_total = 0
    gold_host_total = 0.0
    for c in range(N_CORES):
        bs = slice(c * BC, (c + 1) * BC)
        ids_c = ids[bs]
        tags_c = tags[bs]
        len_c = lengths[bs].astype(np.int64)
        k_len_total += int(np.minimum(len_c, S - 1).sum())

        idx_f = ids_c.T.reshape(-1)                    # token (s, b) order
        idx_b = ids_c[:, ::-1].T.reshape(-1)
        xt = np.stack([gather_xt(idx_f), gather_xt(idx_b)])

        svec = np.arange(S)[None, :]
        valid = (svec < len_c[:, None]).T.reshape(-1)  # [(s, b)]
        ohm = np.zeros((T, NTOK), dtype=ml_dtypes.bfloat16)
        tt = tags_c.T.reshape(-1)
        pos = np.arange(NTOK)
        ohm[tt[valid], pos[valid]] = 1
        vm = np.broadcast_to(valid.astype(ml_dtypes.bfloat16),
                             (T, NTOK)).copy()
        padr = (~valid).astype(ml_dtypes.bfloat16)[None, :]

        # gold-path table part (trans/start/end/b_out counts) on host
        gh = 0.0
        for b in range(BC):
            L = int(len_c[b])
            tg = tags_c[b, :L]
            gh += float(trans[tg[:-1], tg[1:]].sum())
            gh += float(start_t[tg[0]] + end_t[tg[-1]])
            gh += float(bout[tg].sum())
        gold_host_total += gh

        h0t = np.zeros((128, 2, 2, BC), dtype=ml_dtypes.bfloat16)
        c0t = np.zeros((128, 2, 2, BC), dtype=np.float32)
        for d in range(2):
            for k in range(2):
                h0t[:, d, k, :] = h0[d][bs][:, k * 128:(k + 1) * 128].T
                c0t[:, d, k, :] = c0[d][bs][:, k * 128:(k + 1) * 128].T

        in_maps.append(dict(
            xt=xt, wih=wih_pack, whh=whh_pack, biast=bias_pack,
            wout=wout_pack, h0t=h0t, c0t=c0t,
            mp=mp.astype(ml_dtypes.bfloat16),
            mpt=mp.T.copy().astype(ml_dtypes.bfloat16),
            eend=eend, bvec=bvec, ohm=ohm, vmask=vm, padrow=padr,
        ))

    return in_maps, dict(kappa=kappa, k_len_total=k_len_total,
                         gold_host_total=gold_host_total)


def finalize(results, host):
    logz = sum(float(r["out"][0, 0]) for r in results)
    gold_em = sum(float(r["out"][0, 1]) for r in results)
    logz += host["kappa"] * host["k_len_total"]
    score = gold_em + host["gold_host_total"]
    return np.float32((logz - score) / B)


# ---------------------------------------------------------------- entry point
_COMPILED = {}


def kernel(**inputs):
    """Full-input BiLSTM-CRF loss on 8 NeuronCores (data parallel)."""
    from concourse.bass_utils import run_bass_kernel_spmd
    in_maps, host = prep_inputs(inputs)
    if "nc" not in _COMPILED:
        _COMPILED["nc"] = build_nc()
    nc = _COMPILED["nc"]
    res = run_bass_kernel_spmd(nc, in_maps, core_ids=list(range(N_CORES)))
    return np.asarray(finalize(res.results, host))


# revision 24
# speedup vs baseline: 2.2332x; 1.0612x over previous
"""BiLSTM-CRF loss kernel for Trainium2, 8-core data parallel.

Per-core (batch shard of 32), feature-major ("transposed") layout throughout:
gates/features live on partitions, batch on the free dim, so every elementwise
op runs at 128-partition occupancy with a small free size.

  - z_t for each direction accumulates in PSUM as [128 gates-in-chunk,
    8 chunks, batch]: per (chunk, dir) group = 1 bias matmul (K=1 ones rhs)
    + 4 x-projection matmuls (xg in [E, token] layout, consumed in-loop; no
    DRAM z roundtrip) + 2 recurrent matmuls off the transposed h buffer.
  - one sigmoid covers all 8 gate chunks; the g-gate rows of W/b are
    host-prescaled by 2 so tanh(g) = 2*sigmoid(z_g) - 1, done as a single
    DVE scalar_tensor_tensor. f*c runs on GpSimd (Pool) off the DVE path.
  - h = sigma_o * tanh(c) is written directly into the persistent transposed
    h buffer [128, k, dir, token] feeding both the next step's matmuls and
    the emission matmuls -- no PE transposes anywhere.
  - emissions (em = Wout.[hf;hb]) are computed per 512-token block as soon
    as both chains have covered it, with exp/vmask/gold-dot fused in.
  - CRF partition function in scaled linear space with an absorbing 77th
    tag: meet-in-the-middle (alpha forward 64 steps, beta/gamma backward 64
    steps, run concurrently), Z = alpha_63 . (M gamma_64).
Host combines the 8 per-core partial sums into the scalar loss.
"""

import numpy as np
import ml_dtypes

import concourse.bass as bass
import concourse.mybir as mybir
from concourse.tile import TileContext
from concourse.vector_clock import ScopedClock
from concourse.alu_op_type import AluOpType as ALU

N_CORES = 8
B, S, E, HD, T, V = 256, 128, 512, 256, 76, 30000
BC = B // N_CORES          # 32 batch per core
G4 = 4 * HD                # 1024 gates per direction
TA = T + 1                 # 77 tags with absorber
NTOK = S * BC              # 4096 tokens per direction per core

dt = mybir.dt
F32, BF16 = dt.float32, dt.bfloat16
AF = mybir.ActivationFunctionType

# ---------------------------------------------------------------- tile patch
# This walrus build rejects >1 sem wait on CTRL-class (Drain/NoOp)
# instructions; split the Tile tail-drain waits across preceding NOPs.
_MAX_WAITS = 1
_WAIT_LIMITS = {}


def _split_excess_waits(nc):
    """Non-DMA instructions accept only one sem wait on this walrus build;
    move excess waits onto NOPs spliced in front (same engine, same order)."""
    for f in nc.m.functions:
        stack = list(f.blocks)
        while stack:
            bb = stack.pop()
            for sub in getattr(bb, "blocks", []) or []:
                stack.append(sub)
            insts = getattr(bb, "instructions", None)
            if not insts:
                continue
            newlist = []
            changed = False
            for inst in insts:
                si = inst.sync_info
                lim = _WAIT_LIMITS.get(type(inst).__name__, 1)
                if si is not None and si.on_wait and len(si.on_wait) > lim:
                    waits = list(si.on_wait)
                    si.on_wait = waits[-lim:]
                    for w in waits[:-lim]:
                        nop = mybir.InstNoOp(
                            name=f"I-wsplit{nc.next_id()}", ins=[], outs=[],
                            engine=inst.engine,
                            sync_info=mybir.SyncInfo(on_wait=[w], on_update=[]),
                        )
                        newlist.append(nop)
                    changed = True
                newlist.append(inst)
            if changed:
                insts[:] = newlist


def _patched_drain_and_barrier(self, tick_clock, wait_clock):
    nc = self.nc
    _split_excess_waits(nc)
    nops = [nc.sync.nop(nofuse=True, hint=f"waitsplit{i}") for i in range(16)]
    drain_inst = nc.sync.drain()
    wait_clock.add_sem_waits(
        drain_inst.ins, ScopedClock({None: tick_clock.global_clock})
    )
    si = drain_inst.ins.sync_info
    if si is not None and si.on_wait and len(si.on_wait) > _MAX_WAITS:
        waits = list(si.on_wait)
        chunks = [waits[i:i + _MAX_WAITS] for i in range(0, len(waits), _MAX_WAITS)]
        si.on_wait = chunks[-1]
        assert len(chunks) - 1 <= len(nops), "too many wait chunks"
        for i, ch in enumerate(chunks[:-1]):
            ni = nops[i].ins
            if ni.sync_info is None:
                ni.sync_info = mybir.SyncInfo(on_wait=ch, on_update=[])
            else:
                ni.sync_info.on_wait = list(ni.sync_info.on_wait) + ch
    nc.all_engine_barrier()
    assert self.sems is not None
    popped = nc._tile_sem_poison_stack.pop()
    assert popped is self._sem_poison
    allsems = list(self.sems.allocated().values())
    for i in range(0, len(allsems), 8):
        nc.clear_and_free_semaphores(allsems[i:i + 8])
    nc.all_engine_barrier()


def apply_tile_patch():
    TileContext._drain_and_barrier = _patched_drain_and_barrier


# ---------------------------------------------------------------- builder
def build_nc():
    apply_tile_patch()
    nc = bass.Bass("TRN2", target_bir_lowering=False, debug=False,
                   num_devices=N_CORES)

    xt_d = nc.dram_tensor("xt", [2, 128, 4, NTOK], BF16, kind="ExternalInput")
    wih_d = nc.dram_tensor("wih", [128, 2, 4, G4], BF16, kind="ExternalInput")
    whh_d = nc.dram_tensor("whh", [128, 2, 2, G4], BF16, kind="ExternalInput")
    bias_d = nc.dram_tensor("biast", [1, 2, 8, 128], BF16, kind="ExternalInput")
    wout_d = nc.dram_tensor("wout", [128, 2, 2, T], BF16, kind="ExternalInput")
    h0t_d = nc.dram_tensor("h0t", [128, 2, 2, BC], BF16, kind="ExternalInput")
    c0t_d = nc.dram_tensor("c0t", [128, 2, 2, BC], F32, kind="ExternalInput")
    mp_d = nc.dram_tensor("mp", [TA, TA], BF16, kind="ExternalInput")
    mpt_d = nc.dram_tensor("mpt", [TA, TA], BF16, kind="ExternalInput")
    eend_d = nc.dram_tensor("eend", [TA, 1], F32, kind="ExternalInput")
    bvec_d = nc.dram_tensor("bvec", [T, 2], F32, kind="ExternalInput")
    ohm_d = nc.dram_tensor("ohm", [T, NTOK], BF16, kind="ExternalInput")
    vmask_d = nc.dram_tensor("vmask", [T, NTOK], BF16, kind="ExternalInput")
    padrow_d = nc.dram_tensor("padrow", [1, NTOK], BF16, kind="ExternalInput")
    out_d = nc.dram_tensor("out", [1, 2], F32, kind="ExternalOutput")

    NB = S // 16  # 8 emission blocks of 512 tokens
    # slot (0-based) after which emission block b is fully available; the
    # backward chain is software-pipelined one slot behind the forward one
    em_ready = {}
    for b in range(NB):
        r = max(16 * b + 15, S - 16 * b)
        em_ready.setdefault(r, []).append(b)

    with TileContext(nc) as tc:
        with (
            tc.tile_pool(name="const", bufs=1) as cpool,
            tc.tile_pool(name="hbuf", bufs=1) as hpool,
            tc.tile_pool(name="gate", bufs=3) as gpool,
            tc.tile_pool(name="cell", bufs=3) as spool,
            tc.tile_pool(name="work", bufs=3) as wpool,
            tc.tile_pool(name="zps", bufs=2, space="PSUM") as zps_pool,
            tc.tile_pool(name="emps", bufs=1, space="PSUM") as emps_pool,
            tc.tile_pool(name="crfps", bufs=2, space="PSUM") as crfps_pool,
        ):
            # ---- constants / weights into SBUF
            wih_sb = cpool.tile([128, 2, 4, G4], BF16)
            nc.sync.dma_start(wih_sb[:], wih_d[:])
            whh_sb = cpool.tile([128, 2, 2, G4], BF16)
            nc.sync.dma_start(whh_sb[:], whh_d[:])
            bias_sb = cpool.tile([1, 2, 8, 128], BF16)
            nc.sync.dma_start(bias_sb[:], bias_d[:])
            wout_sb = cpool.tile([128, 2, 2, T], BF16)
            nc.sync.dma_start(wout_sb[:], wout_d[:])
            h0t_sb = cpool.tile([128, 2, 2, BC], BF16)
            nc.sync.dma_start(h0t_sb[:], h0t_d[:])
            c0t_sb = cpool.tile([128, 2, 2, BC], F32)
            nc.sync.dma_start(c0t_sb[:], c0t_d[:])
            mp_sb = cpool.tile([TA, TA], BF16)
            nc.sync.dma_start(mp_sb[:], mp_d[:])
            mpt_sb = cpool.tile([TA, TA], BF16)
            nc.sync.dma_start(mpt_sb[:], mpt_d[:])
            eend_sb = cpool.tile([TA, 1], F32)
            nc.sync.dma_start(eend_sb[:], eend_d[:])
            bvec_sb = cpool.tile([T, 2], F32)
            nc.sync.dma_start(bvec_sb[:], bvec_d[:])

            ones1 = cpool.tile([1, BC], BF16)
            nc.vector.memset(ones1[:], 1.0)
            onesd = cpool.tile([128, 2, BC], BF16)
            nc.vector.memset(onesd[:], 1.0)
            ones77 = cpool.tile([TA, 1], F32)
            nc.vector.memset(ones77[:], 1.0)

            # ---- big persistent buffers
            # xg: embeddings in [E-chunk, dir?, ...] layout; chunked DMA so
            # early steps start before the full 8MB lands.
            xg = {d: hpool.tile([128, 4, NTOK], BF16, name=f"xg{d}")
                  for d in range(2)}
            NCH = 4
            CW = NTOK // NCH
            for c in range(NCH):
                for d in range(2):
                    nc.sync.dma_start(
                        xg[d][:, :, c * CW:(c + 1) * CW],
                        xt_d.ap()[d, :, :, c * CW:(c + 1) * CW])
            # transposed h, one tile per direction: [128, k-chunk, token]
            hts = {d: hpool.tile([128, 2, NTOK], BF16, name=f"hts{d}")
                   for d in range(2)}
            # emissions (scaled-exp'd), bf16, absorber row 76
            em_sb = hpool.tile([TA, NTOK], BF16, name="em")
            ohm_sb = hpool.tile([T, NTOK], BF16, name="ohm")
            nc.sync.dma_start(ohm_sb[:], ohm_d[:])
            vm_sb = hpool.tile([T, NTOK], BF16, name="vm")
            nc.sync.dma_start(vm_sb[:], vmask_d[:])
            nc.sync.dma_start(em_sb[T:TA, :], padrow_d[:])

            # ---- z PSUM tile helpers -----------------------------------
            def emit_bias_x(zt, d, t):
                """bias + x-projection matmuls of direction d for step t into
                PSUM tile zt [128, 8 gate-chunk, BC]."""
                tok = slice(t * BC, (t + 1) * BC)
                for gc in range(8):
                    nc.tensor.matmul(
                        zt[:, gc, :], bias_sb[:, d, gc, :],
                        ones1[:], start=True, stop=False)
                for ek in range(4):
                    for gc in range(8):
                        nc.tensor.matmul(
                            zt[:, gc, :],
                            wih_sb[:, d, ek, gc * 128:(gc + 1) * 128],
                            xg[d][:, ek, tok], start=False, stop=False)

            def emit_h(zt, d, t):
                """recurrent matmuls (Whh . h_{t-1}) closing step t's groups."""
                for k in range(2):
                    if t == 0:
                        hk = h0t_sb[:, d, k, :]
                    else:
                        col = (t - 1 if d == 0 else S - t) * BC
                        hk = hts[d][:, k, col:col + BC]
                    for gc in range(8):
                        nc.tensor.matmul(
                            zt[:, gc, :],
                            whh_sb[:, d, k, gc * 128:(gc + 1) * 128],
                            hk, start=False, stop=(k == 1))

            # ---- emission block -----------------------------------------
            em_accs = []

            def emit_emission(b):
                blk = slice(b * 512, (b + 1) * 512)
                ps = emps_pool.tile([T, 512], F32, tag="emps")
                i = 0
                for d in range(2):
                    for k in range(2):
                        nc.tensor.matmul(ps[:], wout_sb[:, k, d, :],
                                         hts[d][:, k, blk],
                                         start=(i == 0), stop=(i == 3))
                        i += 1
                # gold-path dot on raw em (b_out part handled on host)
                acc = wpool.tile([T, 1], F32, tag=f"emacc{b}", bufs=1,
                                 name=f"emacc{b}")
                scr = wpool.tile([T, 512], F32, tag="ttrscr")
                nc.vector.tensor_mul(scr[:], ps[:], ohm_sb[:, blk])
                nc.vector.tensor_reduce(acc[:], scr[:],
                                        axis=mybir.AxisListType.X, op=ALU.add)
                em_accs.append(acc)
                # scaled emissions: exp(em + b_out [+ start on col 0])
                if b == 0:
                    nc.scalar.activation(em_sb[0:T, 0:BC], ps[:, 0:BC],
                                         AF.Exp, bias=bvec_sb[:, 1:2])
                    nc.scalar.activation(em_sb[0:T, BC:512], ps[:, BC:512],
                                         AF.Exp, bias=bvec_sb[:, 0:1])
                else:
                    nc.scalar.activation(em_sb[0:T, blk], ps[:],
                                         AF.Exp, bias=bvec_sb[:, 0:1])
                nc.vector.tensor_mul(em_sb[0:T, blk], em_sb[0:T, blk],
                                     vm_sb[:, blk])

            # ---- LSTM loop ----------------------------------------------
            # Forward chain runs in slot t = its step t; the backward chain
            # is software-pipelined one slot behind (step t in slot t+1) so
            # its Act/DVE ops always have ready inputs and can never stall
            # the forward chain through the in-order engine queues.
            c_st = {d: c0t_sb[:, d, :, :] for d in range(2)}

            def cell_ops(d, t, zt):
                g = gpool.tile([128, 8, BC], BF16, tag=f"g{d}", name=f"g{d}")
                nc.scalar.activation(g[:], zt[:], AF.Sigmoid)
                fc = spool.tile([128, 2, BC], F32, tag=f"fc{d}",
                                name=f"fc{d}")
                nc.vector.tensor_mul(fc[:], g[:, 2:4, :], c_st[d])
                tg = spool.tile([128, 2, BC], BF16, tag=f"tg{d}",
                                name=f"tg{d}")
                # tanh(g) = 2*sigmoid(2g) - 1 (g-rows prescaled by 2)
                nc.vector.scalar_tensor_tensor(
                    tg[:], g[:, 6:8, :], 2.0, onesd[:],
                    op0=ALU.mult, op1=ALU.subtract)
                ig = spool.tile([128, 2, BC], BF16, tag=f"ig{d}",
                                name=f"ig{d}")
                nc.vector.tensor_mul(ig[:], tg[:], g[:, 0:2, :])
                cn = spool.tile([128, 2, BC], F32, tag=f"c{d}", name=f"c{d}")
                nc.vector.tensor_add(cn[:], fc[:], ig[:])
                th = spool.tile([128, 2, BC], BF16, tag=f"th{d}",
                                name=f"th{d}")
                nc.scalar.activation(th[:], cn[:], AF.Tanh)
                col = (t if d == 0 else S - 1 - t) * BC
                nc.vector.tensor_mul(hts[d][:, :, col:col + BC],
                                     g[:, 4:6, :], th[:])
                c_st[d] = cn[:]

            def new_z(d):
                return zps_pool.tile([128, 8, BC], F32, tag=f"z{d}",
                                     name=f"z{d}")

            zcur = {0: new_z(0)}
            emit_bias_x(zcur[0], 0, 0)
            for slot in range(S + 1):
                if slot < S:
                    emit_h(zcur[0], 0, slot)
                    cell_ops(0, slot, zcur[0])
                if slot >= 1:
                    emit_h(zcur[1], 1, slot - 1)
                    cell_ops(1, slot - 1, zcur[1])
                if slot < S - 1:
                    zf = new_z(0)
                    emit_bias_x(zf, 0, slot + 1)
                    zcur[0] = zf
                if slot < S:
                    zb = new_z(1)
                    emit_bias_x(zb, 1, slot)
                    zcur[1] = zb
                for b in em_ready.get(slot, []):
                    emit_emission(b)

            # ---- CRF: meet-in-the-middle forward/backward ---------------
            half = S // 2  # alpha covers em 0..63, gamma covers 127..64
            a_prev = em_sb[:, 0:BC]
            gma = gpool.tile([TA, BC], BF16, tag="gma", name="gma")
            nc.vector.tensor_scalar_mul(
                gma[:], em_sb[:, (S - 1) * BC:S * BC], eend_sb[:])
            g_prev = gma[:]
            for i in range(half - 1):
                ta_ = i + 1
                tb_ = S - 2 - i
                aps = crfps_pool.tile([TA, BC], F32, tag="crf")
                nc.tensor.matmul(aps[:], mp_sb[:], a_prev,
                                 start=True, stop=True)
                a_new = gpool.tile([TA, BC], BF16, tag="a", name="a")
                nc.vector.tensor_mul(
                    a_new[:], aps[:], em_sb[:, ta_ * BC:(ta_ + 1) * BC])
                a_prev = a_new[:]
                gps = crfps_pool.tile([TA, BC], F32, tag="crf")
                nc.tensor.matmul(gps[:], mpt_sb[:], g_prev,
                                 start=True, stop=True)
                g_new = gpool.tile([TA, BC], BF16, tag="gma", name="gma")
                nc.vector.tensor_mul(
                    g_new[:], gps[:], em_sb[:, tb_ * BC:(tb_ + 1) * BC])
                g_prev = g_new[:]

            # Z = alpha_63 . (M gamma_64)
            wps = crfps_pool.tile([TA, BC], F32, tag="crf")
            nc.tensor.matmul(wps[:], mpt_sb[:], g_prev, start=True, stop=True)
            u = wpool.tile([TA, BC], F32, tag="u")
            nc.vector.tensor_mul(u[:], wps[:], a_prev)
            zsc = crfps_pool.tile([1, BC + 8], F32, tag="zsc", bufs=1)
            nc.tensor.matmul(zsc[:, 0:BC], ones77[:], u[:],
                             start=True, stop=True)
            logs = wpool.tile([1, BC], F32, tag="logs")
            nc.scalar.activation(logs[:], zsc[:, 0:BC], AF.Ln)
            logsum = wpool.tile([1, 1], F32, tag="logsum")
            nc.vector.tensor_reduce(logsum[:], logs[:],
                                    axis=mybir.AxisListType.X, op=ALU.add)

            # ---- gold emission score sum --------------------------------
            tot = wpool.tile([T, 1], F32, tag="tot")
            nc.vector.tensor_add(tot[:], em_accs[0][:], em_accs[1][:])
            for acc in em_accs[2:]:
                nc.vector.tensor_add(tot[:], tot[:], acc[:])
            nc.tensor.matmul(zsc[:, BC:BC + 1], ones77[0:T, :], tot[:],
                             start=True, stop=True)

            res = wpool.tile([1, 2], F32, tag="res")
            nc.vector.tensor_copy(res[:, 0:1], logsum[:])
            nc.vector.tensor_copy(res[:, 1:2], zsc[:, BC:BC + 1])
            nc.sync.dma_start(out_d[:], res[:])

    return nc


# ---------------------------------------------------------------- host side
def _gate_perm():
    """PyTorch gate order i,f,g,o -> reordered i,f,o,g (rows of W/b)."""
    return np.concatenate([
        np.arange(0, HD),            # i
        np.arange(HD, 2 * HD),       # f
        np.arange(3 * HD, 4 * HD),   # o
        np.arange(2 * HD, 3 * HD),   # g
    ])


def _pack_w_t(w, perm, nchunks, gscale):
    """w: [G4, kdim] -> [128, nchunks, G4] bf16 with
    out[p, c, g] = w[perm[g], c*128+p] * gscale[g]."""
    wp = np.asarray(w, dtype=np.float32)[perm, :] * gscale[:, None]
    out = np.empty((128, nchunks, G4), dtype=ml_dtypes.bfloat16)
    for c in range(nchunks):
        out[:, c, :] = wp[:, c * 128:(c + 1) * 128].T.astype(ml_dtypes.bfloat16)
    return out


def prep_inputs(inputs):
    """Build per-core input maps + host constants."""
    ids = np.asarray(inputs["input_ids"])
    tags = np.asarray(inputs["tag_ids"])
    lengths = np.asarray(inputs["lengths"])
    perm = _gate_perm()
    # gate g (index 768:1024 after perm) prescaled by 2 for the
    # tanh(x) = 2*sigmoid(2x)-1 identity
    gscale = np.ones(G4, dtype=np.float32)
    gscale[3 * HD:] = 2.0

    embed_bf = np.asarray(inputs["embed_table"]).astype(ml_dtypes.bfloat16)

    def gather_xt(flat_ids):
        g = embed_bf[flat_ids]                       # [NTOK, E] bf16
        return np.ascontiguousarray(
            g.reshape(NTOK, 4, 128).transpose(2, 1, 0))

    wih_pack = np.stack([_pack_w_t(inputs["W_ih_f"], perm, 4, gscale),
                         _pack_w_t(inputs["W_ih_b"], perm, 4, gscale)],
                        axis=1)                      # [128, 2, 4, G4]
    whh_pack = np.stack([_pack_w_t(inputs["W_hh_f"], perm, 2, gscale),
                         _pack_w_t(inputs["W_hh_b"], perm, 2, gscale)],
                        axis=1)                      # [128, 2, 2, G4]
    bias_f = ((np.asarray(inputs["b_ih_f"]) + np.asarray(inputs["b_hh_f"]))
              [perm] * gscale)
    bias_b = ((np.asarray(inputs["b_ih_b"]) + np.asarray(inputs["b_hh_b"]))
              [perm] * gscale)
    bias_pack = np.stack([bias_f.reshape(8, 128), bias_b.reshape(8, 128)]
                         )[None].astype(ml_dtypes.bfloat16)  # [1, 2, 8, 128]

    wo = np.asarray(inputs["W_out"])                 # [T, H]
    wout_pack = np.empty((128, 2, 2, T), dtype=ml_dtypes.bfloat16)
    for d in range(2):
        for k in range(2):
            sl = slice(d * 256 + k * 128, d * 256 + (k + 1) * 128)
            wout_pack[:, k, d, :] = wo[:, sl].T.astype(ml_dtypes.bfloat16)

    trans = np.asarray(inputs["trans"]).astype(np.float64)
    start_t = np.asarray(inputs["start_trans"]).astype(np.float64)
    end_t = np.asarray(inputs["end_trans"]).astype(np.float64)
    bout = np.asarray(inputs["b_out"]).astype(np.float64)
    kappa = float(np.log(np.exp(trans).sum(axis=0).mean()))

    mp = np.zeros((TA, TA), dtype=np.float64)
    mp[0:T, 0:T] = np.exp(trans - kappa)
    mp[0:T, T] = np.exp(end_t - kappa)
    mp[T, T] = 1.0
    eend = np.zeros((TA, 1), dtype=np.float32)
    eend[0:T, 0] = np.exp(end_t)
    eend[T, 0] = 1.0
    bvec = np.zeros((T, 2), dtype=np.float32)
    bvec[:, 0] = bout
    bvec[:, 1] = bout + start_t

    h0 = np.asarray(inputs["h0"])                    # [2, B, HD]
    c0 = np.asarray(inputs["c0"])

    in_maps = []
    k_len_total = 0
    gold_host_total = 0.0
    for c in range(N_CORES):
        bs = slice(c * BC, (c + 1) * BC)
        ids_c = ids[bs]
        tags_c = tags[bs]
        len_c = lengths[bs].astype(np.int64)
        k_len_total += int(np.minimum(len_c, S - 1).sum())

        idx_f = ids_c.T.reshape(-1)                    # token (s, b) order
        idx_b = ids_c[:, ::-1].T.reshape(-1)
        xt = np.stack([gather_xt(idx_f), gather_xt(idx_b)])

        svec = np.arange(S)[None, :]
        valid = (svec < len_c[:, None]).T.reshape(-1)  # [(s, b)]
        ohm = np.zeros((T, NTOK), dtype=ml_dtypes.bfloat16)
        tt = tags_c.T.reshape(-1)
        pos = np.arange(NTOK)
        ohm[tt[valid], pos[valid]] = 1
        vm = np.broadcast_to(valid.astype(ml_dtypes.bfloat16),
                             (T, NTOK)).copy()
        padr = (~valid).astype(ml_dtypes.bfloat16)[None, :]

        # gold-path table part (trans/start/end/b_out counts) on host
        gh = 0.0
        for b in range(BC):
            L = int(len_c[b])
            tg = tags_c[b, :L]
            gh += float(trans[tg[:-1], tg[1:]].sum())
            gh += float(start_t[tg[0]] + end_t[tg[-1]])
            gh += float(bout[tg].sum())
        gold_host_total += gh

        h0t = np.zeros((128, 2, 2, BC), dtype=ml_dtypes.bfloat16)
        c0t = np.zeros((128, 2, 2, BC), dtype=np.float32)
        for d in range(2):
            for k in range(2):
                h0t[:, d, k, :] = h0[d][bs][:, k * 128:(k + 1) * 128].T
                c0t[:, d, k, :] = c0[d][bs][:, k * 128:(k + 1) * 128].T

        in_maps.append(dict(
            xt=xt, wih=wih_pack, whh=whh_pack, biast=bias_pack,
            wout=wout_pack, h0t=h0t, c0t=c0t,
            mp=mp.astype(ml_dtypes.bfloat16),
            mpt=mp.T.copy().astype(ml_dtypes.bfloat16),
            eend=eend, bvec=bvec, ohm=ohm, vmask=vm, padrow=padr,
        ))

    return in_maps, dict(kappa=kappa, k_len_total=k_len_total,
                         gold_host_total=gold_host_total)


def finalize(results, host):
    logz = sum(float(r["out"][0, 0]) for r in results)
    gold_em = sum(float(r["out"][0, 1]) for r in results)
    logz += host["kappa"] * host["k_len_total"]
    score = gold_em + host["gold_host_total"]
    return np.float32((logz - score) / B)


# ---------------------------------------------------------------- entry point
_COMPILED = {}


def kernel(**inputs):
    """Full-input BiLSTM-CRF loss on 8 NeuronCores (data parallel)."""
    from concourse.bass_utils import run_bass_kernel_spmd
    in_maps, host = prep_inputs(inputs)
    if "nc" not in _COMPILED:
        _COMPILED["nc"] = build_nc()
    nc = _COMPILED["nc"]
    res = run_bass_kernel_spmd(nc, in_maps, core_ids=list(range(N_CORES)))
    return np.asarray(finalize(res.results, host))
